# revision 9
# baseline (speedup 1.0000x reference)
"""Trainium2 Bass kernel for MultiHeadLatentAttention (B=2, T=2048, C=2048, 16 heads).

Sharding over 8 NeuronCores: core c = (batch b = c//4, r = c%4).
 - Latent projections (x@wq_a, x@wkv_a) computed token-sharded (quarter r),
   in transposed layout (latent-dim on partitions), then AllGather-ed within
   each 4-core batch group (each gather split in two halves so consumers can
   start earlier).
 - Each core then handles head-group r (4 of 16 heads) for the full sequence:
   up-projections, RoPE+RMSNorm, block-causal attention, and a row-shard of
   the output projection.  Host sums the 4 partial outputs per batch.

All matmuls bf16 with fp32 PSUM accumulation.  RMS/softmax denominators are
ones-matmul partition reductions batched into 32-aligned PSUM rows (one DVE
reciprocal per four rows); per-row broadcasts are selector-matmuls.  Phases
are emitted interleaved (K/V sections, then Q sections with attention blocks
woven between them, then the output projection with the last attention block's
tail hidden under it) so the PE's in-order stream never waits on the
vector/scalar-engine chains and the ScalarE-bound softmax overlaps PE-dense
projections.
"""

from contextlib import ExitStack

import numpy as np
import ml_dtypes

import concourse.bass as bass
import concourse.tile as tile
import concourse.mybir as mybir
from concourse import bacc
from concourse.bass_utils import run_bass_kernel_spmd

BF16 = mybir.dt.bfloat16
F32 = mybir.dt.float32
NPBF16 = ml_dtypes.bfloat16
AF = mybir.ActivationFunctionType

P = 128
B, T, C = 2, 2048, 2048
H, D = 16, 128
LORA = 1024
KV_PE = 256           # latent rows 0-255 (chunks 0-1)
CONTENT = 768         # latent rows 256-1023 (chunks 2-7)
EPS = 1.1920929e-07
HG = 4                # heads per core
TQ = 512              # tokens per quarter / query block
NLB = LORA // P       # 8 latent row-blocks
NCC = C // P          # 16 contraction chunks of x
NTT = T // TQ         # 4 token 512-tiles
NKT = T // P          # 16 key tiles of 128
NQB = T // TQ         # 4 query blocks of 512
RG = [[0, 1, 2, 3], [4, 5, 6, 7]]

USE_AG = True

_NC_CACHE = {}


def build_nc(use_ag=USE_AG):
    nc = bacc.Bacc("TRN2", target_bir_lowering=False, debug=False, num_devices=8)

    xT = nc.dram_tensor("xT", [C, TQ if use_ag else T], BF16, kind="ExternalInput")
    wq_a = nc.dram_tensor("wq_a", [C, LORA], BF16, kind="ExternalInput")
    wkv_a = nc.dram_tensor("wkv_a", [C, LORA], BF16, kind="ExternalInput")
    wq_b = nc.dram_tensor("wq_b", [LORA, HG * D], BF16, kind="ExternalInput")
    wk_b = nc.dram_tensor("wk_b", [CONTENT, HG * D], BF16, kind="ExternalInput")
    wkpe_b = nc.dram_tensor("wkpe_b", [KV_PE, HG * D], BF16, kind="ExternalInput")
    wv_b = nc.dram_tensor("wv_b", [CONTENT, HG * D], BF16, kind="ExternalInput")
    wo = nc.dram_tensor("wo", [HG * D, C], BF16, kind="ExternalInput")
    # duplicated rope tables: cos2 = [cos; cos], sin2n = [sin; -sin]
    cos2 = nc.dram_tensor("cos2", [P, T], BF16, kind="ExternalInput")
    sin2n = nc.dram_tensor("sin2n", [P, T], BF16, kind="ExternalInput")
    tri = nc.dram_tensor("tri", [P, P], BF16, kind="ExternalInput")
    outT = nc.dram_tensor("outT", [C, T], F32, kind="ExternalOutput")

    with tile.TileContext(nc) as tc, ExitStack() as ctx:
        dram = ctx.enter_context(tc.tile_pool(name="dram", bufs=1, space="DRAM"))
        psum = ctx.enter_context(tc.tile_pool(name="psum", bufs=8, space="PSUM"))
        consts = ctx.enter_context(tc.tile_pool(name="consts", bufs=1))
        persist = ctx.enter_context(tc.tile_pool(name="persist", bufs=1))
        tmpk = ctx.enter_context(tc.tile_pool(name="tmpk", bufs=8))
        ropep = ctx.enter_context(tc.tile_pool(name="ropep", bufs=6))
        tmpsq = ctx.enter_context(tc.tile_pool(name="tmpsq", bufs=4))
        normf = ctx.enter_context(tc.tile_pool(name="normf", bufs=3))
        normb = ctx.enter_context(tc.tile_pool(name="normb", bufs=4))
        expool = ctx.enter_context(tc.tile_pool(name="expool", bufs=6))
        accpool = ctx.enter_context(tc.tile_pool(name="accpool", bufs=4))
        castpool = ctx.enter_context(tc.tile_pool(name="castpool", bufs=18))

        def ps_tile(name):
            return psum.tile([P, 512], F32, name=name, tag="ps")

        def row_mm(out_tile, h, lhsT, rhs):
            # ones-matmul partition reduction into 32-aligned row 32*h.
            # Each row-MM is its own complete accumulation group: the rows are
            # disjoint so Tile sees no deps between them and may reorder; a
            # shared group would then accumulate onto stale bank bits.
            tp = (0, 32 * h) if h == 3 else None
            nc.tensor.matmul(out_tile[32 * h:32 * h + 1, :], lhsT, rhs,
                             start=True, stop=True, tile_position=tp)

        # ---- constants ----
        cos2_sb = consts.tile([P, T], BF16, name="cos2_sb")
        sin2n_sb = consts.tile([P, T], BF16, name="sin2n_sb")
        nc.sync.dma_start(out=cos2_sb[:], in_=cos2[:])
        nc.sync.dma_start(out=sin2n_sb[:], in_=sin2n[:])
        tri_sb = consts.tile([P, P], BF16, name="tri_sb")
        nc.sync.dma_start(out=tri_sb[:], in_=tri[:])
        ones_red = consts.tile([P, 1], BF16, name="ones_red")
        nc.vector.memset(ones_red[:], 1.0)
        zeros128 = consts.tile([P, 1], F32, name="zeros128")
        nc.vector.memset(zeros128[:], 0.0)
        eps_k128 = consts.tile([P, 1], F32, name="eps_k128")
        nc.vector.memset(eps_k128[:], EPS)
        eps_q128 = consts.tile([P, 1], F32, name="eps_q128")
        nc.vector.memset(eps_q128[:], float(D) * EPS)
        sels = []
        for j in range(4):
            s = consts.tile([P, P], BF16, name=f"sel{j}")
            nc.vector.memset(s[:], 0.0)
            nc.vector.memset(s[32 * j:32 * j + 1, :], 1.0)
            sels.append(s)

        # ---- persistent phase products ----
        yTn_sb = persist.tile([P, HG, T], BF16, name="yTn_sb")
        # attention inputs live only until the last attention block; their
        # pool closes before phase O's pools open so the space is reused
        attn_ctx = ExitStack()
        attnp = attn_ctx.enter_context(tc.tile_pool(name="attnp", bufs=1))
        kTn_sb = attnp.tile([P, HG, T], BF16, name="kTn_sb")
        qTn_sb = attnp.tile([P, HG, T], BF16, name="qTn_sb")
        v_sb = attnp.tile([P, NKT, HG * D], BF16, name="v_sb")

        # up-projection weights (resident until end of Q sections)
        wu = attn_ctx.enter_context(tc.tile_pool(name="wu", bufs=1))
        wkb_sb = wu.tile([P, CONTENT // P, HG * D], BF16, name="wkb_sb")
        nc.sync.dma_start(out=wkb_sb[:], in_=wk_b.rearrange("(j p) n -> p j n", p=P))
        wkpe_sb = wu.tile([P, KV_PE // P, HG * D], BF16, name="wkpe_sb")
        nc.sync.dma_start(out=wkpe_sb[:], in_=wkpe_b.rearrange("(j p) n -> p j n", p=P))
        wv_sb = wu.tile([P, CONTENT // P, HG * D], BF16, name="wv_sb")
        nc.sync.dma_start(out=wv_sb[:], in_=wv_b.rearrange("(j p) n -> p j n", p=P))
        wqb_sb = wu.tile([P, NLB, HG * D], BF16, name="wqb_sb")
        nc.sync.dma_start(out=wqb_sb[:], in_=wq_b.rearrange("(j p) n -> p j n", p=P))

        # ---- phase L: latent projections + (halved) AllGathers ----
        # halves by latent rows: kv: A = chunks 2-5 (content head), B = chunks
        # 0,1,6,7 (pe + content tail); q: A = chunks 0-3, B = 4-7.
        KV_HALF_A = [2, 3, 4, 5]
        KV_HALF_B = [0, 1, 6, 7]
        Q_HALF_A = [0, 1, 2, 3]
        Q_HALF_B = [4, 5, 6, 7]
        cc_out = {}
        if use_ag:
            with tc.tile_pool(name="xpool", bufs=16) as xpool, \
                 tc.tile_pool(name="wstream", bufs=4) as wsp, \
                 tc.tile_pool(name="latstage", bufs=2) as lsp:
                xsb = []
                for cc in range(NCC):
                    t = xpool.tile([P, TQ], BF16, name=f"xsb{cc}", tag="xsb")
                    nc.sync.dma_start(out=t[:], in_=xT[cc * P:(cc + 1) * P, :])
                    xsb.append(t)
                for wname, wh, lbs in [
                    ("kva", wkv_a, KV_HALF_A), ("kvb", wkv_a, KV_HALF_B),
                    ("qa", wq_a, Q_HALF_A), ("qb", wq_a, Q_HALF_B),
                ]:
                    ccin = dram.tile([4 * P, TQ], BF16, name=f"cc_in_{wname}",
                                     tag=f"cc_in_{wname}")
                    ccout = dram.tile([16 * P, TQ], BF16, name=f"cc_out_{wname}",
                                      tag=f"cc_out_{wname}")
                    cc_out[wname] = ccout
                    lat = lsp.tile([P, 4, TQ], BF16, name=f"lat_{wname}", tag="lat")
                    pss = [ps_tile(f"lat_ps_{wname}{i}") for i in range(4)]
                    for cc in range(NCC):
                        wt = wsp.tile([P, 4, P], BF16, name=f"wt_{wname}{cc}", tag="wt")
                        src = wh[cc * P:(cc + 1) * P, :].rearrange(
                            "p (c q) -> p c q", q=P)
                        for i, lb in enumerate(lbs):
                            nc.sync.dma_start(out=wt[:, i, :], in_=src[:, lb, :])
                        for i in range(4):
                            nc.tensor.matmul(
                                pss[i][:], wt[:, i, :], xsb[cc][:],
                                start=(cc == 0), stop=(cc == NCC - 1))
                    for i in range(4):
                        nc.scalar.copy(out=lat[:, i, :], in_=pss[i][:])
                    for i in range(4):
                        nc.sync.dma_start(out=ccin[i * P:(i + 1) * P, :], in_=lat[:, i, :])
                    nc.gpsimd.collective_compute(
                        "AllGather", mybir.AluOpType.bypass, replica_groups=RG,
                        ins=[ccin.opt()], outs=[ccout.opt()])

        def load_lat(pool, name, tt, half_a, half_b, names):
            # assemble the 8-chunk latent block for token-tile tt from the two
            # gathered halves (or compute locally when use_ag=False)
            t = pool.tile([P, NLB, TQ], BF16, name=name, tag=pool.name)
            for src_name, lbs in ((names[0], half_a), (names[1], half_b)):
                ccout = cc_out[src_name]
                blk = ccout[4 * P * tt:4 * P * (tt + 1), :].rearrange(
                    "(c p) t -> p c t", p=P)
                for i, lb in enumerate(lbs):
                    nc.sync.dma_start(out=t[:, lb, :], in_=blk[:, i, :])
            return t

        # ---- K/V sections ----
        def k_head(h, tt, kvsb_t, ss_k, kuns):
            kc_ps = ps_tile(f"kc_ps_{h}_{tt}")
            for j in range(CONTENT // P):
                nc.tensor.matmul(kc_ps[:], wkb_sb[:, j, h * D:(h + 1) * D],
                                 kvsb_t[:, 2 + j, :], start=(j == 0), stop=(j == 5))
            kpe_ps = ps_tile(f"kpe_ps_{h}_{tt}")
            for j in range(KV_PE // P):
                nc.tensor.matmul(kpe_ps[:], wkpe_sb[:, j, h * D:(h + 1) * D],
                                 kvsb_t[:, j, :], start=(j == 0), stop=(j == 1))
            hd = D // 2
            # kswap = halves of kpe swapped (PSUM reads may cross partitions)
            kswap = ropep.tile([P, TQ], BF16, name=f"kswap_{h}_{tt}", tag="rope")
            nc.scalar.copy(out=kswap[0:hd, :], in_=kpe_ps[hd:D, :])
            nc.scalar.copy(out=kswap[hd:D, :], in_=kpe_ps[0:hd, :])
            t1 = ropep.tile([P, TQ], BF16, name=f"t1_{h}_{tt}", tag="rope")
            nc.vector.tensor_mul(t1[:], kpe_ps[:], cos2_sb[:, tt * TQ:(tt + 1) * TQ])
            t2 = ropep.tile([P, TQ], BF16, name=f"t2_{h}_{tt}", tag="rope")
            nc.vector.tensor_mul(t2[:], kswap[:], sin2n_sb[:, tt * TQ:(tt + 1) * TQ])
            nc.vector.tensor_add(t1[:], t1[:], t2[:])
            k_un = tmpk.tile([P, TQ], BF16, name=f"k_un_{h}_{tt}", tag="k_un")
            nc.vector.tensor_add(k_un[:], t1[:], kc_ps[:])
            kuns.append(k_un)
            sq = tmpsq.tile([P, TQ], BF16, name=f"ksq_{h}_{tt}", tag="sq")
            nc.vector.tensor_mul(sq[:], k_un[:], k_un[:])
            row_mm(ss_k, h, ones_red[:], sq[:])

        def v_block(tt, t4, kvsb_t):
            v_ps = ps_tile(f"v_ps_{tt}_{t4}")
            for j in range(CONTENT // P):
                nc.tensor.matmul(v_ps[:], kvsb_t[:, 2 + j, t4 * P:(t4 + 1) * P],
                                 wv_sb[:, j, :], start=(j == 0), stop=(j == 5))
            nc.scalar.copy(out=v_sb[:, tt * 4 + t4, :], in_=v_ps[:])

        def norm_tail(tt, ss, scale, bias_t, srcs, dst, which):
            sroot = normf.tile([P, TQ], F32, name=f"sroot_{which}_{tt}", tag="nf")
            nc.scalar.activation(sroot[:], ss[:], AF.Sqrt, bias=bias_t[:], scale=scale)
            rinv = normf.tile([P, TQ], F32, name=f"rinv_{which}_{tt}", tag="nf")
            nc.vector.reciprocal(rinv[:], sroot[:])
            rbf = normb.tile([P, TQ], BF16, name=f"rbf_{which}_{tt}", tag="nb")
            nc.vector.tensor_copy(out=rbf[:], in_=rinv[:])
            for h in range(HG):
                bc = ps_tile(f"bc_{which}_{h}_{tt}")
                nc.tensor.matmul(bc[:], sels[h][:], rbf[:], start=True, stop=True)
                nc.vector.tensor_mul(dst[:, h, tt * TQ:(tt + 1) * TQ], srcs[h][:], bc[:])

        def q_sec(tt, qlsb_t, ss_q, qcs):
            for h in range(HG):
                q_ps = ps_tile(f"q_ps_{h}_{tt}")
                for j in range(NLB):
                    nc.tensor.matmul(q_ps[:], wqb_sb[:, j, h * D:(h + 1) * D],
                                     qlsb_t[:, j, :], start=(j == 0), stop=(j == NLB - 1))
                qc = castpool.tile([P, TQ], BF16, name=f"qc_{h}_{tt}", tag="cast")
                nc.scalar.copy(out=qc[:], in_=q_ps[:])
                qcs.append(qc)
                sq = tmpsq.tile([P, TQ], BF16, name=f"qsq_{h}_{tt}", tag="sq")
                nc.scalar.activation(sq[:], q_ps[:], AF.Square, bias=zeros128[:], scale=1.0)
                row_mm(ss_q, h, ones_red[:], sq[:])

        # ---- attention ----
        def a_tail(qb, den4, ycs):
            rinv = normf.tile([P, TQ], F32, name=f"rden_{qb}", tag="nf")
            nc.vector.reciprocal(rinv[:], den4[:])
            rbf = normb.tile([P, TQ], BF16, name=f"rdenb_{qb}", tag="nb")
            nc.vector.tensor_copy(out=rbf[:], in_=rinv[:])
            for h in range(HG):
                bc = ps_tile(f"abc_{h}_{qb}")
                nc.tensor.matmul(bc[:], sels[h][:], rbf[:], start=True, stop=True)
                nc.vector.tensor_mul(yTn_sb[:, h, qb * TQ:(qb + 1) * TQ],
                                     ycs[h][:], bc[:])

        pending_a = []

        def a_block(qb):
            # memset to 1.0 (not 0): unused rows go through reciprocal and
            # 1/0=inf would poison the selector matmul with 0*inf=NaN
            den4 = ps_tile(f"den4_{qb}")
            nc.vector.memset(den4[:], 1.0)
            ycs = []
            nkt = 4 * (qb + 1)
            for h in range(HG):
                yt_ps = ps_tile(f"yt_ps_{h}_{qb}")
                acc = accpool.tile([P, TQ], BF16, name=f"acc_{h}_{qb}", tag="acc")
                for kt in range(nkt):
                    sc_ps = ps_tile(f"sc_ps_{h}_{qb}_{kt}")
                    nc.tensor.matmul(sc_ps[:], kTn_sb[:, h, kt * P:(kt + 1) * P],
                                     qTn_sb[:, h, qb * TQ:(qb + 1) * TQ],
                                     start=True, stop=True)
                    ex = expool.tile([P, TQ], BF16, name=f"ex_{h}_{qb}_{kt}", tag="ex")
                    nc.scalar.activation(ex[:], sc_ps[:], AF.Exp,
                                         bias=zeros128[:], scale=1.0)
                    jrel = kt - 4 * qb
                    if jrel >= 0:
                        if jrel > 0:
                            nc.vector.memset(ex[:, 0:P * jrel], 0.0)
                        nc.vector.tensor_mul(ex[:, P * jrel:P * (jrel + 1)],
                                             ex[:, P * jrel:P * (jrel + 1)], tri_sb[:])
                    if kt == 0:
                        nc.vector.tensor_copy(out=acc[:], in_=ex[:])
                    else:
                        nc.vector.tensor_add(acc[:], acc[:], ex[:])
                    nc.tensor.matmul(yt_ps[:], v_sb[:, kt, h * D:(h + 1) * D], ex[:],
                                     start=(kt == 0), stop=(kt == nkt - 1))
                row_mm(den4, h, ones_red[:], acc[:])
                yc = castpool.tile([P, TQ], BF16, name=f"yc_{h}_{qb}", tag="cast")
                nc.scalar.copy(out=yc[:], in_=yt_ps[:])
                ycs.append(yc)
                if pending_a and h == 1:
                    a_tail(*pending_a.pop(0))
            pending_a.append((qb, den4, ycs))

        # ---- emission: KV sections, then Q sections woven with A blocks ----
        with tc.tile_pool(name="kvpool", bufs=2) as kvpool, \
             tc.tile_pool(name="qlpool", bufs=2) as qlpool, \
             tc.tile_pool(name="xpool2", bufs=16) as xpool2, \
             tc.tile_pool(name="wstream2", bufs=3) as wsp2:

            def local_lat(pool, name, tt, wh):
                dst = pool.tile([P, NLB, TQ], BF16, name=name, tag=pool.name)
                xsb2 = []
                for cc in range(NCC):
                    t = xpool2.tile([P, TQ], BF16, name=f"x2_{name}_{cc}", tag="xsb2")
                    nc.sync.dma_start(out=t[:], in_=xT[cc * P:(cc + 1) * P,
                                                      tt * TQ:(tt + 1) * TQ])
                    xsb2.append(t)
                pss = [ps_tile(f"lat_ps_{name}_{lb}") for lb in range(NLB)]
                for cc in range(NCC):
                    wt = wsp2.tile([P, LORA], BF16, name=f"w2_{name}_{cc}", tag="wt2")
                    nc.sync.dma_start(out=wt[:], in_=wh[cc * P:(cc + 1) * P, :])
                    for lb in range(NLB):
                        nc.tensor.matmul(pss[lb][:], wt[:, lb * P:(lb + 1) * P],
                                         xsb2[cc][:], start=(cc == 0), stop=(cc == NCC - 1))
                for lb in range(NLB):
                    nc.scalar.copy(out=dst[:, lb, :], in_=pss[lb][:])
                return dst

            pending_k = []
            for tt in range(NTT):
                if use_ag:
                    kvsb_t = load_lat(kvpool, f"kvsb{tt}", tt, KV_HALF_A, KV_HALF_B,
                                      ("kva", "kvb"))
                else:
                    kvsb_t = local_lat(kvpool, f"kvsb{tt}", tt, wkv_a)
                ss_k = ps_tile(f"ss_k_{tt}")
                nc.vector.memset(ss_k[:], 1.0)
                kuns = []
                for h in range(HG):
                    k_head(h, tt, kvsb_t, ss_k, kuns)
                if pending_k:
                    p = pending_k.pop(0)
                    norm_tail(p[0], p[1], 1.0 / D, eps_k128, p[2], kTn_sb, "k")
                for t4 in range(4):
                    v_block(tt, t4, kvsb_t)
                pending_k.append((tt, ss_k, kuns))
            p = pending_k.pop(0)
            norm_tail(p[0], p[1], 1.0 / D, eps_k128, p[2], kTn_sb, "k")

            pending_q = []
            for tt in range(NTT):
                if use_ag:
                    qlsb_t = load_lat(qlpool, f"qlsb{tt}", tt, Q_HALF_A, Q_HALF_B,
                                      ("qa", "qb"))
                else:
                    qlsb_t = local_lat(qlpool, f"qlsb{tt}", tt, wq_a)
                ss_q = ps_tile(f"ss_q_{tt}")
                nc.vector.memset(ss_q[:], 1.0)
                qcs = []
                q_sec(tt, qlsb_t, ss_q, qcs)
                if pending_q:
                    p = pending_q.pop(0)
                    norm_tail(p[0], p[1], 1.0, eps_q128, p[2], qTn_sb, "q")
                    a_block(p[0])
                pending_q.append((tt, ss_q, qcs))
            p = pending_q.pop(0)
            norm_tail(p[0], p[1], 1.0, eps_q128, p[2], qTn_sb, "q")
            a_block(p[0])

        # ---- phase O: output projection; pass 1 (tt 0-2) can run while the
        # last attention block's ScalarE work drains, then tail + pass 2 ----
        attn_ctx.close()
        opool = ctx.enter_context(tc.tile_pool(name="opool", bufs=4))
        wop = ctx.enter_context(tc.tile_pool(name="wop", bufs=16))
        wo_ts = []
        for ct in range(C // P):
            wo_t = wop.tile([P, HG, P], BF16, name=f"wo_t{ct}", tag="wo_t")
            nc.sync.dma_start(out=wo_t[:],
                              in_=wo[:, ct * P:(ct + 1) * P].rearrange("(h p) c -> p h c", p=P))
            wo_ts.append(None)
            ops = [ps_tile(f"o_ps_{ct}_{tt}") for tt in range(3)]
            for h in range(HG):
                for tt in range(3):
                    nc.tensor.matmul(ops[tt][:], wo_t[:, h, :],
                                     yTn_sb[:, h, tt * TQ:(tt + 1) * TQ],
                                     start=(h == 0), stop=(h == HG - 1))
            for tt in range(3):
                o_sb = opool.tile([P, TQ], F32, name=f"o_sb_{ct}_{tt}", tag="o_sb")
                nc.vector.tensor_copy(out=o_sb[:], in_=ops[tt][:])
                nc.sync.dma_start(out=outT[ct * P:(ct + 1) * P, tt * TQ:(tt + 1) * TQ],
                                  in_=o_sb[:])
            if ct == 0:
                a_tail(*pending_a.pop(0))
            wo_ts[ct] = wo_t

        for ct in range(C // P):
            o_ps = ps_tile(f"o_ps3_{ct}")
            for h in range(HG):
                nc.tensor.matmul(o_ps[:], wo_ts[ct][:, h, :],
                                 yTn_sb[:, h, 3 * TQ:4 * TQ],
                                 start=(h == 0), stop=(h == HG - 1))
            o_sb = opool.tile([P, TQ], F32, name=f"o_sb3_{ct}", tag="o_sb")
            nc.vector.tensor_copy(out=o_sb[:], in_=o_ps[:])
            nc.sync.dma_start(out=outT[ct * P:(ct + 1) * P, 3 * TQ:4 * TQ], in_=o_sb[:])

    nc.compile()
    return nc


def _get_nc(use_ag=USE_AG):
    if use_ag not in _NC_CACHE:
        _NC_CACHE[use_ag] = build_nc(use_ag)
    return _NC_CACHE[use_ag]


def _prepare_in_maps(x, cos, sin, wq_a, wq_b, wkv_a, wk_b, wkpe_b, wv_b, wo, use_ag=USE_AG):
    def bf(a):
        return np.ascontiguousarray(a).astype(NPBF16)

    cosT = np.asarray(cos, np.float32)[0, :, 0, :].T   # (64, T)
    sinT = np.asarray(sin, np.float32)[0, :, 0, :].T
    cos2 = bf(np.concatenate([cosT, cosT], axis=0))    # (128, T)
    sin2n = bf(np.concatenate([sinT, -sinT], axis=0))
    tri = (np.arange(P)[:, None] <= np.arange(P)[None, :]).astype(NPBF16)

    wq_a_b, wkv_a_b = bf(wq_a), bf(wkv_a)
    wq_b_b, wk_b_b = bf(wq_b), bf(wk_b)
    wkpe_b_b, wv_b_b, wo_b = bf(wkpe_b), bf(wv_b), bf(wo)
    x = np.asarray(x, np.float32)

    in_maps = []
    for c in range(8):
        b, r = c // 4, c % 4
        if use_ag:
            xT_c = bf(x[b, r * TQ:(r + 1) * TQ, :].T)
        else:
            xT_c = bf(x[b].T)
        hgs = slice(r * HG * D, (r + 1) * HG * D)
        in_maps.append({
            "xT": xT_c,
            "wq_a": wq_a_b,
            "wkv_a": wkv_a_b,
            "wq_b": np.ascontiguousarray(wq_b_b[:, hgs]),
            "wk_b": np.ascontiguousarray(wk_b_b[:, hgs]),
            "wkpe_b": np.ascontiguousarray(wkpe_b_b[:, hgs]),
            "wv_b": np.ascontiguousarray(wv_b_b[:, hgs]),
            "wo": np.ascontiguousarray(wo_b[hgs, :]),
            "cos2": cos2,
            "sin2n": sin2n,
            "tri": tri,
        })
    return in_maps


def _assemble(results):
    out = np.empty((B, T, C), np.float32)
    for b in range(B):
        acc = results[4 * b]["outT"].astype(np.float32).copy()
        for r in range(1, 4):
            acc += results[4 * b + r]["outT"]
        out[b] = acc.T
    return out


def _run(inputs, use_ag=USE_AG, trace=False):
    nc = _get_nc(use_ag)
    in_maps = _prepare_in_maps(use_ag=use_ag, **inputs)
    res = run_bass_kernel_spmd(nc, in_maps, core_ids=list(range(8)), trace=trace)
    return _assemble(res.results), res


def kernel(**inputs):
    out, _ = _run(inputs)
    return out


# revision 11
# speedup vs baseline: 1.1208x; 1.1208x over previous
"""Trainium2 Bass kernel for MultiHeadLatentAttention (B=2, T=2048, C=2048, 16 heads).

Sharding over 8 NeuronCores: core c = (batch b = c//4, r = c%4).
 - Latent projections (x@wq_a, x@wkv_a) computed token-sharded (quarter r),
   in transposed layout (latent-dim on partitions), then AllGather-ed within
   each 4-core batch group (each gather split in two halves so consumers can
   start earlier).
 - Each core then handles head-group r (4 of 16 heads) for the full sequence:
   up-projections, RoPE+RMSNorm, block-causal attention, and a row-shard of
   the output projection.  Host sums the 4 partial outputs per batch.

All matmuls bf16 with fp32 PSUM accumulation.  RMS/softmax denominators are
ones-matmul partition reductions batched into 32-aligned PSUM rows (one DVE
reciprocal per four rows); per-row broadcasts are selector-matmuls.  Phases
are emitted interleaved (K/V sections, then Q sections with attention blocks
woven between them, then the output projection with the last attention block's
tail hidden under it) so the PE's in-order stream never waits on the
vector/scalar-engine chains and the ScalarE-bound softmax overlaps PE-dense
projections.
"""

from contextlib import ExitStack

import numpy as np
import ml_dtypes

import concourse.bass as bass
import concourse.tile as tile
import concourse.mybir as mybir
from concourse import bacc
from concourse.bass_utils import run_bass_kernel_spmd

BF16 = mybir.dt.bfloat16
F32 = mybir.dt.float32
NPBF16 = ml_dtypes.bfloat16
AF = mybir.ActivationFunctionType

P = 128
B, T, C = 2, 2048, 2048
H, D = 16, 128
LORA = 1024
KV_PE = 256           # latent rows 0-255 (chunks 0-1)
CONTENT = 768         # latent rows 256-1023 (chunks 2-7)
EPS = 1.1920929e-07
HG = 4                # heads per core
TQ = 512              # tokens per quarter / query block
NLB = LORA // P       # 8 latent row-blocks
NCC = C // P          # 16 contraction chunks of x
NTT = T // TQ         # 4 token 512-tiles
NKT = T // P          # 16 key tiles of 128
NQB = T // TQ         # 4 query blocks of 512
RG = [[0, 1, 2, 3], [4, 5, 6, 7]]

USE_AG = True

_NC_CACHE = {}


def build_nc(use_ag=USE_AG):
    nc = bacc.Bacc("TRN2", target_bir_lowering=False, debug=False, num_devices=8)

    xT = nc.dram_tensor("xT", [C, TQ if use_ag else T], BF16, kind="ExternalInput")
    wq_a = nc.dram_tensor("wq_a", [C, LORA], BF16, kind="ExternalInput")
    wkv_a = nc.dram_tensor("wkv_a", [C, LORA], BF16, kind="ExternalInput")
    wq_b = nc.dram_tensor("wq_b", [LORA, HG * D], BF16, kind="ExternalInput")
    wk_b = nc.dram_tensor("wk_b", [CONTENT, HG * D], BF16, kind="ExternalInput")
    wkpe_b = nc.dram_tensor("wkpe_b", [KV_PE, HG * D], BF16, kind="ExternalInput")
    wv_b = nc.dram_tensor("wv_b", [CONTENT, HG * D], BF16, kind="ExternalInput")
    wo = nc.dram_tensor("wo", [HG * D, C], BF16, kind="ExternalInput")
    # duplicated rope tables: cos2 = [cos; cos], sin2n = [sin; -sin]
    cos2 = nc.dram_tensor("cos2", [P, T], BF16, kind="ExternalInput")
    sin2n = nc.dram_tensor("sin2n", [P, T], BF16, kind="ExternalInput")
    tri = nc.dram_tensor("tri", [P, P], BF16, kind="ExternalInput")
    outT = nc.dram_tensor("outT", [C, T], F32, kind="ExternalOutput")

    with tile.TileContext(nc) as tc, ExitStack() as ctx:
        dram = ctx.enter_context(tc.tile_pool(name="dram", bufs=1, space="DRAM"))
        psum = ctx.enter_context(tc.tile_pool(name="psum", bufs=8, space="PSUM"))
        consts = ctx.enter_context(tc.tile_pool(name="consts", bufs=1))
        persist = ctx.enter_context(tc.tile_pool(name="persist", bufs=1))
        tmpk = ctx.enter_context(tc.tile_pool(name="tmpk", bufs=8))
        ropep = ctx.enter_context(tc.tile_pool(name="ropep", bufs=6))
        tmpsq = ctx.enter_context(tc.tile_pool(name="tmpsq", bufs=4))
        normf = ctx.enter_context(tc.tile_pool(name="normf", bufs=3))
        normb = ctx.enter_context(tc.tile_pool(name="normb", bufs=4))
        expool = ctx.enter_context(tc.tile_pool(name="expool", bufs=6))
        accpool = ctx.enter_context(tc.tile_pool(name="accpool", bufs=4))
        castpool = ctx.enter_context(tc.tile_pool(name="castpool", bufs=18))

        def ps_tile(name):
            return psum.tile([P, 512], F32, name=name, tag="ps")

        def row_mm(out_tile, h, lhsT, rhs):
            # ones-matmul partition reduction into 32-aligned row 32*h.
            # Each row-MM is its own complete accumulation group: the rows are
            # disjoint so Tile sees no deps between them and may reorder; a
            # shared group would then accumulate onto stale bank bits.
            tp = (0, 32 * h) if h == 3 else None
            nc.tensor.matmul(out_tile[32 * h:32 * h + 1, :], lhsT, rhs,
                             start=True, stop=True, tile_position=tp)

        # ---- constants ----
        cos2_sb = consts.tile([P, T], BF16, name="cos2_sb")
        sin2n_sb = consts.tile([P, T], BF16, name="sin2n_sb")
        nc.sync.dma_start(out=cos2_sb[:], in_=cos2[:])
        nc.sync.dma_start(out=sin2n_sb[:], in_=sin2n[:])
        tri_sb = consts.tile([P, P], BF16, name="tri_sb")
        nc.sync.dma_start(out=tri_sb[:], in_=tri[:])
        ones_red = consts.tile([P, 1], BF16, name="ones_red")
        nc.vector.memset(ones_red[:], 1.0)
        zeros128 = consts.tile([P, 1], F32, name="zeros128")
        nc.vector.memset(zeros128[:], 0.0)
        eps_k128 = consts.tile([P, 1], F32, name="eps_k128")
        nc.vector.memset(eps_k128[:], EPS)
        eps_q128 = consts.tile([P, 1], F32, name="eps_q128")
        nc.vector.memset(eps_q128[:], float(D) * EPS)
        sels = []
        for j in range(4):
            s = consts.tile([P, P], BF16, name=f"sel{j}")
            nc.vector.memset(s[:], 0.0)
            nc.vector.memset(s[32 * j:32 * j + 1, :], 1.0)
            sels.append(s)

        # ---- persistent phase products ----
        yTn_sb = persist.tile([P, HG, T], BF16, name="yTn_sb")
        # attention inputs live only until the last attention block; their
        # pool closes before phase O's pools open so the space is reused
        attn_ctx = ExitStack()
        attnp = attn_ctx.enter_context(tc.tile_pool(name="attnp", bufs=1))
        kTn_sb = attnp.tile([P, HG, T], BF16, name="kTn_sb")
        qTn_sb = attnp.tile([P, HG, T], BF16, name="qTn_sb")
        v_sb = attnp.tile([P, NKT, HG * D], BF16, name="v_sb")

        # up-projection weights (resident until end of Q sections)
        wu = attn_ctx.enter_context(tc.tile_pool(name="wu", bufs=1))
        wkb_sb = wu.tile([P, CONTENT // P, HG * D], BF16, name="wkb_sb")
        nc.sync.dma_start(out=wkb_sb[:], in_=wk_b.rearrange("(j p) n -> p j n", p=P))
        wkpe_sb = wu.tile([P, KV_PE // P, HG * D], BF16, name="wkpe_sb")
        nc.sync.dma_start(out=wkpe_sb[:], in_=wkpe_b.rearrange("(j p) n -> p j n", p=P))
        wv_sb = wu.tile([P, CONTENT // P, HG * D], BF16, name="wv_sb")
        nc.sync.dma_start(out=wv_sb[:], in_=wv_b.rearrange("(j p) n -> p j n", p=P))
        wqb_sb = wu.tile([P, NLB, HG * D], BF16, name="wqb_sb")
        nc.sync.dma_start(out=wqb_sb[:], in_=wq_b.rearrange("(j p) n -> p j n", p=P))

        # ---- phase L: latent projections + (halved) AllGathers ----
        # halves by latent rows: kv: A = chunks 2-5 (content head), B = chunks
        # 0,1,6,7 (pe + content tail); q: A = chunks 0-3, B = 4-7.
        KV_HALF_A = [2, 3, 4, 5]
        KV_HALF_B = [0, 1, 6, 7]
        Q_HALF_A = [0, 1, 2, 3]
        Q_HALF_B = [4, 5, 6, 7]
        cc_out = {}
        if use_ag:
            with tc.tile_pool(name="xpool", bufs=16) as xpool, \
                 tc.tile_pool(name="wstream", bufs=4) as wsp, \
                 tc.tile_pool(name="latstage", bufs=2) as lsp:
                xsb = []
                for cc in range(NCC):
                    t = xpool.tile([P, TQ], BF16, name=f"xsb{cc}", tag="xsb")
                    nc.sync.dma_start(out=t[:], in_=xT[cc * P:(cc + 1) * P, :])
                    xsb.append(t)
                for wname, wh, lbs in [
                    ("kva", wkv_a, KV_HALF_A), ("kvb", wkv_a, KV_HALF_B),
                    ("qa", wq_a, Q_HALF_A), ("qb", wq_a, Q_HALF_B),
                ]:
                    ccin = dram.tile([4 * P, TQ], BF16, name=f"cc_in_{wname}",
                                     tag=f"cc_in_{wname}")
                    ccout = dram.tile([16 * P, TQ], BF16, name=f"cc_out_{wname}",
                                      tag=f"cc_out_{wname}")
                    cc_out[wname] = ccout
                    lat = lsp.tile([P, 4, TQ], BF16, name=f"lat_{wname}", tag="lat")
                    pss = [ps_tile(f"lat_ps_{wname}{i}") for i in range(4)]
                    half = 0 if wname in ("kva", "qa") else 1
                    for cc in range(NCC):
                        # host permuted the weight columns into half order, so
                        # each half is one contiguous 512-column slab
                        wt = wsp.tile([P, 4 * P], BF16, name=f"wt_{wname}{cc}", tag="wt")
                        nc.sync.dma_start(
                            out=wt[:],
                            in_=wh[cc * P:(cc + 1) * P, half * 4 * P:(half + 1) * 4 * P])
                        for i in range(4):
                            nc.tensor.matmul(
                                pss[i][:], wt[:, i * P:(i + 1) * P], xsb[cc][:],
                                start=(cc == 0), stop=(cc == NCC - 1))
                    for i in range(4):
                        nc.scalar.copy(out=lat[:, i, :], in_=pss[i][:])
                    for i in range(4):
                        nc.sync.dma_start(out=ccin[i * P:(i + 1) * P, :], in_=lat[:, i, :])
                    nc.gpsimd.collective_compute(
                        "AllGather", mybir.AluOpType.bypass, replica_groups=RG,
                        ins=[ccin.opt()], outs=[ccout.opt()])

        def load_lat(pool, name, tt, half_a, half_b, names):
            # assemble the 8-chunk latent block for token-tile tt from the two
            # gathered halves (or compute locally when use_ag=False)
            t = pool.tile([P, NLB, TQ], BF16, name=name, tag=pool.name)
            for src_name, lbs in ((names[0], half_a), (names[1], half_b)):
                ccout = cc_out[src_name]
                blk = ccout[4 * P * tt:4 * P * (tt + 1), :].rearrange(
                    "(c p) t -> p c t", p=P)
                for i, lb in enumerate(lbs):
                    nc.sync.dma_start(out=t[:, lb, :], in_=blk[:, i, :])
            return t

        # ---- K/V sections ----
        def k_head(h, tt, kvsb_t, ss_k, kuns):
            kc_ps = ps_tile(f"kc_ps_{h}_{tt}")
            for j in range(CONTENT // P):
                nc.tensor.matmul(kc_ps[:], wkb_sb[:, j, h * D:(h + 1) * D],
                                 kvsb_t[:, 2 + j, :], start=(j == 0), stop=(j == 5))
            kpe_ps = ps_tile(f"kpe_ps_{h}_{tt}")
            for j in range(KV_PE // P):
                nc.tensor.matmul(kpe_ps[:], wkpe_sb[:, j, h * D:(h + 1) * D],
                                 kvsb_t[:, j, :], start=(j == 0), stop=(j == 1))
            hd = D // 2
            # kswap = halves of kpe swapped (PSUM reads may cross partitions)
            kswap = ropep.tile([P, TQ], BF16, name=f"kswap_{h}_{tt}", tag="rope")
            nc.scalar.copy(out=kswap[0:hd, :], in_=kpe_ps[hd:D, :])
            nc.scalar.copy(out=kswap[hd:D, :], in_=kpe_ps[0:hd, :])
            t1 = ropep.tile([P, TQ], BF16, name=f"t1_{h}_{tt}", tag="rope")
            nc.vector.tensor_mul(t1[:], kpe_ps[:], cos2_sb[:, tt * TQ:(tt + 1) * TQ])
            t2 = ropep.tile([P, TQ], BF16, name=f"t2_{h}_{tt}", tag="rope")
            nc.vector.tensor_mul(t2[:], kswap[:], sin2n_sb[:, tt * TQ:(tt + 1) * TQ])
            nc.vector.tensor_add(t1[:], t1[:], t2[:])
            k_un = tmpk.tile([P, TQ], BF16, name=f"k_un_{h}_{tt}", tag="k_un")
            nc.vector.tensor_add(k_un[:], t1[:], kc_ps[:])
            kuns.append(k_un)
            sq = tmpsq.tile([P, TQ], BF16, name=f"ksq_{h}_{tt}", tag="sq")
            nc.vector.tensor_mul(sq[:], k_un[:], k_un[:])
            row_mm(ss_k, h, ones_red[:], sq[:])

        def v_block(tt, t4, kvsb_t):
            v_ps = ps_tile(f"v_ps_{tt}_{t4}")
            for j in range(CONTENT // P):
                nc.tensor.matmul(v_ps[:], kvsb_t[:, 2 + j, t4 * P:(t4 + 1) * P],
                                 wv_sb[:, j, :], start=(j == 0), stop=(j == 5))
            nc.scalar.copy(out=v_sb[:, tt * 4 + t4, :], in_=v_ps[:])

        def norm_tail(tt, ss, scale, bias_t, srcs, dst, which):
            sroot = normf.tile([P, TQ], F32, name=f"sroot_{which}_{tt}", tag="nf")
            nc.scalar.activation(sroot[:], ss[:], AF.Sqrt, bias=bias_t[:], scale=scale)
            rinv = normf.tile([P, TQ], F32, name=f"rinv_{which}_{tt}", tag="nf")
            nc.vector.reciprocal(rinv[:], sroot[:])
            rbf = normb.tile([P, TQ], BF16, name=f"rbf_{which}_{tt}", tag="nb")
            nc.vector.tensor_copy(out=rbf[:], in_=rinv[:])
            for h in range(HG):
                bc = ps_tile(f"bc_{which}_{h}_{tt}")
                nc.tensor.matmul(bc[:], sels[h][:], rbf[:], start=True, stop=True)
                nc.vector.tensor_mul(dst[:, h, tt * TQ:(tt + 1) * TQ], srcs[h][:], bc[:])

        def q_sec(tt, qlsb_t, ss_q, qcs):
            qps = []
            for h in range(HG):
                q_ps = ps_tile(f"q_ps_{h}_{tt}")
                qps.append(q_ps)
                for j in range(NLB // 2):
                    nc.tensor.matmul(q_ps[:], wqb_sb[:, j, h * D:(h + 1) * D],
                                     qlsb_t[:, j, :], start=(j == 0), stop=False)
            for h in range(HG):
                q_ps = qps[h]
                for j in range(NLB // 2, NLB):
                    nc.tensor.matmul(q_ps[:], wqb_sb[:, j, h * D:(h + 1) * D],
                                     qlsb_t[:, j, :], start=False, stop=(j == NLB - 1))
                qc = castpool.tile([P, TQ], BF16, name=f"qc_{h}_{tt}", tag="cast")
                nc.scalar.copy(out=qc[:], in_=q_ps[:])
                qcs.append(qc)
                sq = tmpsq.tile([P, TQ], BF16, name=f"qsq_{h}_{tt}", tag="sq")
                nc.scalar.activation(sq[:], q_ps[:], AF.Square, bias=zeros128[:], scale=1.0)
                row_mm(ss_q, h, ones_red[:], sq[:])

        # ---- attention ----
        def a_tail(qb, den4, ycs):
            rinv = normf.tile([P, TQ], F32, name=f"rden_{qb}", tag="nf")
            nc.vector.reciprocal(rinv[:], den4[:])
            rbf = normb.tile([P, TQ], BF16, name=f"rdenb_{qb}", tag="nb")
            nc.vector.tensor_copy(out=rbf[:], in_=rinv[:])
            for h in range(HG):
                bc = ps_tile(f"abc_{h}_{qb}")
                nc.tensor.matmul(bc[:], sels[h][:], rbf[:], start=True, stop=True)
                nc.vector.tensor_mul(yTn_sb[:, h, qb * TQ:(qb + 1) * TQ],
                                     ycs[h][:], bc[:])

        pending_a = []

        def a_block(qb):
            # memset to 1.0 (not 0): unused rows go through reciprocal and
            # 1/0=inf would poison the selector matmul with 0*inf=NaN
            den4 = ps_tile(f"den4_{qb}")
            nc.vector.memset(den4[:], 1.0)
            ycs = []
            nkt = 4 * (qb + 1)
            for h in range(HG):
                yt_ps = ps_tile(f"yt_ps_{h}_{qb}")
                acc = accpool.tile([P, TQ], BF16, name=f"acc_{h}_{qb}", tag="acc")
                for kt in range(nkt):
                    sc_ps = ps_tile(f"sc_ps_{h}_{qb}_{kt}")
                    nc.tensor.matmul(sc_ps[:], kTn_sb[:, h, kt * P:(kt + 1) * P],
                                     qTn_sb[:, h, qb * TQ:(qb + 1) * TQ],
                                     start=True, stop=True)
                    ex = expool.tile([P, TQ], BF16, name=f"ex_{h}_{qb}_{kt}", tag="ex")
                    nc.scalar.activation(ex[:], sc_ps[:], AF.Exp,
                                         bias=zeros128[:], scale=1.0)
                    jrel = kt - 4 * qb
                    if jrel >= 0:
                        if jrel > 0:
                            nc.vector.memset(ex[:, 0:P * jrel], 0.0)
                        nc.vector.tensor_mul(ex[:, P * jrel:P * (jrel + 1)],
                                             ex[:, P * jrel:P * (jrel + 1)], tri_sb[:])
                    if kt == 0:
                        nc.vector.tensor_copy(out=acc[:], in_=ex[:])
                    else:
                        nc.vector.tensor_add(acc[:], acc[:], ex[:])
                    nc.tensor.matmul(yt_ps[:], v_sb[:, kt, h * D:(h + 1) * D], ex[:],
                                     start=(kt == 0), stop=(kt == nkt - 1))
                row_mm(den4, h, ones_red[:], acc[:])
                yc = castpool.tile([P, TQ], BF16, name=f"yc_{h}_{qb}", tag="cast")
                nc.scalar.copy(out=yc[:], in_=yt_ps[:])
                ycs.append(yc)
                if pending_a and h == 1:
                    a_tail(*pending_a.pop(0))
            pending_a.append((qb, den4, ycs))

        # ---- emission: KV sections, then Q sections woven with A blocks ----
        with tc.tile_pool(name="kvpool", bufs=2) as kvpool, \
             tc.tile_pool(name="qlpool", bufs=2) as qlpool, \
             tc.tile_pool(name="xpool2", bufs=16) as xpool2, \
             tc.tile_pool(name="wstream2", bufs=3) as wsp2:

            def local_lat(pool, name, tt, wh, order):
                dst = pool.tile([P, NLB, TQ], BF16, name=name, tag=pool.name)
                xsb2 = []
                for cc in range(NCC):
                    t = xpool2.tile([P, TQ], BF16, name=f"x2_{name}_{cc}", tag="xsb2")
                    nc.sync.dma_start(out=t[:], in_=xT[cc * P:(cc + 1) * P,
                                                      tt * TQ:(tt + 1) * TQ])
                    xsb2.append(t)
                pss = [ps_tile(f"lat_ps_{name}_{lb}") for lb in range(NLB)]
                for cc in range(NCC):
                    wt = wsp2.tile([P, LORA], BF16, name=f"w2_{name}_{cc}", tag="wt2")
                    nc.sync.dma_start(out=wt[:], in_=wh[cc * P:(cc + 1) * P, :])
                    for lb in range(NLB):
                        nc.tensor.matmul(pss[lb][:], wt[:, lb * P:(lb + 1) * P],
                                         xsb2[cc][:], start=(cc == 0), stop=(cc == NCC - 1))
                for pos, lb in enumerate(order):
                    nc.scalar.copy(out=dst[:, lb, :], in_=pss[pos][:])
                return dst

            pending_k = []
            for tt in range(NTT):
                if use_ag:
                    kvsb_t = load_lat(kvpool, f"kvsb{tt}", tt, KV_HALF_A, KV_HALF_B,
                                      ("kva", "kvb"))
                else:
                    kvsb_t = local_lat(kvpool, f"kvsb{tt}", tt, wkv_a, KV_HALF_A + KV_HALF_B)
                ss_k = ps_tile(f"ss_k_{tt}")
                nc.vector.memset(ss_k[:], 1.0)
                kuns = []
                for h in range(HG):
                    k_head(h, tt, kvsb_t, ss_k, kuns)
                if pending_k:
                    p = pending_k.pop(0)
                    norm_tail(p[0], p[1], 1.0 / D, eps_k128, p[2], kTn_sb, "k")
                for t4 in range(4):
                    v_block(tt, t4, kvsb_t)
                pending_k.append((tt, ss_k, kuns))
            p = pending_k.pop(0)
            norm_tail(p[0], p[1], 1.0 / D, eps_k128, p[2], kTn_sb, "k")

            pending_q = []
            for tt in range(NTT):
                if use_ag:
                    qlsb_t = load_lat(qlpool, f"qlsb{tt}", tt, Q_HALF_A, Q_HALF_B,
                                      ("qa", "qb"))
                else:
                    qlsb_t = local_lat(qlpool, f"qlsb{tt}", tt, wq_a, Q_HALF_A + Q_HALF_B)
                ss_q = ps_tile(f"ss_q_{tt}")
                nc.vector.memset(ss_q[:], 1.0)
                qcs = []
                q_sec(tt, qlsb_t, ss_q, qcs)
                if pending_q:
                    p = pending_q.pop(0)
                    norm_tail(p[0], p[1], 1.0, eps_q128, p[2], qTn_sb, "q")
                    a_block(p[0])
                pending_q.append((tt, ss_q, qcs))
            p = pending_q.pop(0)
            norm_tail(p[0], p[1], 1.0, eps_q128, p[2], qTn_sb, "q")
            a_block(p[0])

        # ---- phase O: output projection; pass 1 (tt 0-2) can run while the
        # last attention block's ScalarE work drains, then tail + pass 2 ----
        attn_ctx.close()
        opool = ctx.enter_context(tc.tile_pool(name="opool", bufs=4))
        wop = ctx.enter_context(tc.tile_pool(name="wop", bufs=16))
        wo_ts = []
        for ct in range(C // P):
            wo_t = wop.tile([P, HG, P], BF16, name=f"wo_t{ct}", tag="wo_t")
            nc.sync.dma_start(out=wo_t[:],
                              in_=wo[:, ct * P:(ct + 1) * P].rearrange("(h p) c -> p h c", p=P))
            wo_ts.append(None)
            ops = [ps_tile(f"o_ps_{ct}_{tt}") for tt in range(3)]
            for h in range(HG):
                for tt in range(3):
                    nc.tensor.matmul(ops[tt][:], wo_t[:, h, :],
                                     yTn_sb[:, h, tt * TQ:(tt + 1) * TQ],
                                     start=(h == 0), stop=(h == HG - 1))
            for tt in range(3):
                o_sb = opool.tile([P, TQ], F32, name=f"o_sb_{ct}_{tt}", tag="o_sb")
                nc.vector.tensor_copy(out=o_sb[:], in_=ops[tt][:])
                nc.sync.dma_start(out=outT[ct * P:(ct + 1) * P, tt * TQ:(tt + 1) * TQ],
                                  in_=o_sb[:])
            if ct == 0:
                a_tail(*pending_a.pop(0))
            wo_ts[ct] = wo_t

        for ct in range(C // P):
            o_ps = ps_tile(f"o_ps3_{ct}")
            for h in range(HG):
                nc.tensor.matmul(o_ps[:], wo_ts[ct][:, h, :],
                                 yTn_sb[:, h, 3 * TQ:4 * TQ],
                                 start=(h == 0), stop=(h == HG - 1))
            o_sb = opool.tile([P, TQ], F32, name=f"o_sb3_{ct}", tag="o_sb")
            nc.vector.tensor_copy(out=o_sb[:], in_=o_ps[:])
            nc.sync.dma_start(out=outT[ct * P:(ct + 1) * P, 3 * TQ:4 * TQ], in_=o_sb[:])

    nc.compile()
    return nc


def _get_nc(use_ag=USE_AG):
    if use_ag not in _NC_CACHE:
        _NC_CACHE[use_ag] = build_nc(use_ag)
    return _NC_CACHE[use_ag]


def _prepare_in_maps(x, cos, sin, wq_a, wq_b, wkv_a, wk_b, wkpe_b, wv_b, wo, use_ag=USE_AG):
    def bf(a):
        return np.ascontiguousarray(a).astype(NPBF16)

    cosT = np.asarray(cos, np.float32)[0, :, 0, :].T   # (64, T)
    sinT = np.asarray(sin, np.float32)[0, :, 0, :].T
    cos2 = bf(np.concatenate([cosT, cosT], axis=0))    # (128, T)
    sin2n = bf(np.concatenate([sinT, -sinT], axis=0))
    tri = (np.arange(P)[:, None] <= np.arange(P)[None, :]).astype(NPBF16)

    # permute latent-projection output columns into AllGather-half order so
    # the kernel streams contiguous 512-column slabs per half
    def perm_cols(w, halves):
        idx = np.concatenate([np.arange(c * P, (c + 1) * P) for half in halves for c in half])
        return np.ascontiguousarray(np.asarray(w, np.float32)[:, idx])
    KV_HALVES = ([2, 3, 4, 5], [0, 1, 6, 7])
    Q_HALVES = ([0, 1, 2, 3], [4, 5, 6, 7])
    wq_a_b = bf(perm_cols(wq_a, Q_HALVES))
    wkv_a_b = bf(perm_cols(wkv_a, KV_HALVES))
    wq_b_b, wk_b_b = bf(wq_b), bf(wk_b)
    wkpe_b_b, wv_b_b, wo_b = bf(wkpe_b), bf(wv_b), bf(wo)
    x = np.asarray(x, np.float32)

    in_maps = []
    for c in range(8):
        b, r = c // 4, c % 4
        if use_ag:
            xT_c = bf(x[b, r * TQ:(r + 1) * TQ, :].T)
        else:
            xT_c = bf(x[b].T)
        hgs = slice(r * HG * D, (r + 1) * HG * D)
        in_maps.append({
            "xT": xT_c,
            "wq_a": wq_a_b,
            "wkv_a": wkv_a_b,
            "wq_b": np.ascontiguousarray(wq_b_b[:, hgs]),
            "wk_b": np.ascontiguousarray(wk_b_b[:, hgs]),
            "wkpe_b": np.ascontiguousarray(wkpe_b_b[:, hgs]),
            "wv_b": np.ascontiguousarray(wv_b_b[:, hgs]),
            "wo": np.ascontiguousarray(wo_b[hgs, :]),
            "cos2": cos2,
            "sin2n": sin2n,
            "tri": tri,
        })
    return in_maps


def _assemble(results):
    out = np.empty((B, T, C), np.float32)
    for b in range(B):
        acc = results[4 * b]["outT"].astype(np.float32).copy()
        for r in range(1, 4):
            acc += results[4 * b + r]["outT"]
        out[b] = acc.T
    return out


def _run(inputs, use_ag=USE_AG, trace=False):
    nc = _get_nc(use_ag)
    in_maps = _prepare_in_maps(use_ag=use_ag, **inputs)
    res = run_bass_kernel_spmd(nc, in_maps, core_ids=list(range(8)), trace=trace)
    return _assemble(res.results), res


def kernel(**inputs):
    out, _ = _run(inputs)
    return out


# revision 12
# speedup vs baseline: 1.2049x; 1.0750x over previous
"""Trainium2 Bass kernel for MultiHeadLatentAttention (B=2, T=2048, C=2048, 16 heads).

Sharding over 8 NeuronCores: core c = (batch b = c//4, r = c%4).
 - Latent projections (x@wq_a, x@wkv_a) computed token-sharded (quarter r),
   in transposed layout (latent-dim on partitions), then AllGather-ed within
   each 4-core batch group (each gather split in two halves so consumers can
   start earlier).
 - Each core then handles head-group r (4 of 16 heads) for the full sequence:
   up-projections, RoPE+RMSNorm, block-causal attention, and a row-shard of
   the output projection.  Host sums the 4 partial outputs per batch.

All matmuls bf16 with fp32 PSUM accumulation.  RMS/softmax denominators are
ones-matmul partition reductions batched into 32-aligned PSUM rows (one DVE
reciprocal per four rows); per-row broadcasts are selector-matmuls.  Phases
are emitted interleaved (K/V sections, then Q sections with attention blocks
woven between them, then the output projection with the last attention block's
tail hidden under it) so the PE's in-order stream never waits on the
vector/scalar-engine chains and the ScalarE-bound softmax overlaps PE-dense
projections.
"""

from contextlib import ExitStack

import numpy as np
import ml_dtypes

import concourse.bass as bass
import concourse.tile as tile
import concourse.mybir as mybir
from concourse import bacc
from concourse.bass_utils import run_bass_kernel_spmd

BF16 = mybir.dt.bfloat16
F32 = mybir.dt.float32
NPBF16 = ml_dtypes.bfloat16
AF = mybir.ActivationFunctionType

P = 128
B, T, C = 2, 2048, 2048
H, D = 16, 128
LORA = 1024
KV_PE = 256           # latent rows 0-255 (chunks 0-1)
CONTENT = 768         # latent rows 256-1023 (chunks 2-7)
EPS = 1.1920929e-07
HG = 4                # heads per core
TQ = 512              # tokens per quarter / query block
NLB = LORA // P       # 8 latent row-blocks
NCC = C // P          # 16 contraction chunks of x
NTT = T // TQ         # 4 token 512-tiles
NKT = T // P          # 16 key tiles of 128
NQB = T // TQ         # 4 query blocks of 512
RG = [[0, 1, 2, 3], [4, 5, 6, 7]]

USE_AG = True

_NC_CACHE = {}


def build_nc(use_ag=USE_AG):
    nc = bacc.Bacc("TRN2", target_bir_lowering=False, debug=False, num_devices=8)

    xT = nc.dram_tensor("xT", [C, TQ if use_ag else T], BF16, kind="ExternalInput")
    wq_a = nc.dram_tensor("wq_a", [C, LORA], BF16, kind="ExternalInput")
    wkv_a = nc.dram_tensor("wkv_a", [C, LORA], BF16, kind="ExternalInput")
    wq_b = nc.dram_tensor("wq_b", [LORA, HG * D], BF16, kind="ExternalInput")
    wk_b = nc.dram_tensor("wk_b", [CONTENT, HG * D], BF16, kind="ExternalInput")
    wkpe_b = nc.dram_tensor("wkpe_b", [KV_PE, HG * D], BF16, kind="ExternalInput")
    wv_b = nc.dram_tensor("wv_b", [CONTENT, HG * D], BF16, kind="ExternalInput")
    wo = nc.dram_tensor("wo", [HG * D, C], BF16, kind="ExternalInput")
    # duplicated rope tables: cos2 = [cos; cos], sin2n = [sin; -sin]
    cos2 = nc.dram_tensor("cos2", [P, T], BF16, kind="ExternalInput")
    sin2n = nc.dram_tensor("sin2n", [P, T], BF16, kind="ExternalInput")
    tri = nc.dram_tensor("tri", [P, P], BF16, kind="ExternalInput")
    outT = nc.dram_tensor("outT", [C, T], BF16, kind="ExternalOutput")

    with tile.TileContext(nc) as tc, ExitStack() as ctx:
        dram = ctx.enter_context(tc.tile_pool(name="dram", bufs=1, space="DRAM"))
        psum = ctx.enter_context(tc.tile_pool(name="psum", bufs=8, space="PSUM"))
        consts = ctx.enter_context(tc.tile_pool(name="consts", bufs=1))
        persist = ctx.enter_context(tc.tile_pool(name="persist", bufs=1))
        tmpk = ctx.enter_context(tc.tile_pool(name="tmpk", bufs=8))
        ropep = ctx.enter_context(tc.tile_pool(name="ropep", bufs=6))
        tmpsq = ctx.enter_context(tc.tile_pool(name="tmpsq", bufs=4))
        normf = ctx.enter_context(tc.tile_pool(name="normf", bufs=3))
        normb = ctx.enter_context(tc.tile_pool(name="normb", bufs=4))
        expool = ctx.enter_context(tc.tile_pool(name="expool", bufs=6))
        accpool = ctx.enter_context(tc.tile_pool(name="accpool", bufs=4))
        castpool = ctx.enter_context(tc.tile_pool(name="castpool", bufs=18))

        def ps_tile(name):
            return psum.tile([P, 512], F32, name=name, tag="ps")

        def row_mm(out_tile, h, lhsT, rhs):
            # ones-matmul partition reduction into 32-aligned row 32*h.
            # Each row-MM is its own complete accumulation group: the rows are
            # disjoint so Tile sees no deps between them and may reorder; a
            # shared group would then accumulate onto stale bank bits.
            tp = (0, 32 * h) if h == 3 else None
            nc.tensor.matmul(out_tile[32 * h:32 * h + 1, :], lhsT, rhs,
                             start=True, stop=True, tile_position=tp)

        # ---- warm-up primer: ~4us of dense matmuls on locally-memset tiles
        # (no DMA deps) so the PE's HAM clock gate is released before the
        # first real matmuls arrive ----
        prime_sb = consts.tile([P, TQ], BF16, name="prime_sb")
        nc.vector.memset(prime_sb[:], 0.001)
        prime_w = consts.tile([P, P], BF16, name="prime_w")
        nc.vector.memset(prime_w[:], 0.001)
        prime_ps = ps_tile("prime_ps")
        for i in range(18):
            nc.tensor.matmul(prime_ps[:], prime_w[:], prime_sb[:],
                             start=(i == 0), stop=(i == 17))

        # ---- constants ----
        cos2_sb = consts.tile([P, T], BF16, name="cos2_sb")
        sin2n_sb = consts.tile([P, T], BF16, name="sin2n_sb")
        nc.scalar.dma_start(out=cos2_sb[:], in_=cos2[:])
        nc.scalar.dma_start(out=sin2n_sb[:], in_=sin2n[:])
        tri_sb = consts.tile([P, P], BF16, name="tri_sb")
        nc.scalar.dma_start(out=tri_sb[:], in_=tri[:])
        ones_red = consts.tile([P, 1], BF16, name="ones_red")
        nc.vector.memset(ones_red[:], 1.0)
        zeros128 = consts.tile([P, 1], F32, name="zeros128")
        nc.vector.memset(zeros128[:], 0.0)
        eps_k128 = consts.tile([P, 1], F32, name="eps_k128")
        nc.vector.memset(eps_k128[:], EPS)
        eps_q128 = consts.tile([P, 1], F32, name="eps_q128")
        nc.vector.memset(eps_q128[:], float(D) * EPS)
        sels = []
        for j in range(4):
            s = consts.tile([P, P], BF16, name=f"sel{j}")
            nc.vector.memset(s[:], 0.0)
            nc.vector.memset(s[32 * j:32 * j + 1, :], 1.0)
            sels.append(s)

        # ---- persistent phase products ----
        yTn_sb = persist.tile([P, HG, T], BF16, name="yTn_sb")
        # attention inputs live only until the last attention block; their
        # pool closes before phase O's pools open so the space is reused
        attn_ctx = ExitStack()
        attnp = attn_ctx.enter_context(tc.tile_pool(name="attnp", bufs=1))
        kTn_sb = attnp.tile([P, HG, T], BF16, name="kTn_sb")
        qTn_sb = attnp.tile([P, HG, T], BF16, name="qTn_sb")
        v_sb = attnp.tile([P, NKT, HG * D], BF16, name="v_sb")

        # up-projection weights (resident until end of Q sections)
        wu = attn_ctx.enter_context(tc.tile_pool(name="wu", bufs=1))
        wkb_sb = wu.tile([P, CONTENT // P, HG * D], BF16, name="wkb_sb")
        nc.scalar.dma_start(out=wkb_sb[:], in_=wk_b.rearrange("(j p) n -> p j n", p=P))
        wkpe_sb = wu.tile([P, KV_PE // P, HG * D], BF16, name="wkpe_sb")
        nc.scalar.dma_start(out=wkpe_sb[:], in_=wkpe_b.rearrange("(j p) n -> p j n", p=P))
        wv_sb = wu.tile([P, CONTENT // P, HG * D], BF16, name="wv_sb")
        nc.scalar.dma_start(out=wv_sb[:], in_=wv_b.rearrange("(j p) n -> p j n", p=P))
        wqb_sb = wu.tile([P, NLB, HG * D], BF16, name="wqb_sb")
        nc.scalar.dma_start(out=wqb_sb[:], in_=wq_b.rearrange("(j p) n -> p j n", p=P))

        # ---- phase L: latent projections + (halved) AllGathers ----
        # halves by latent rows: kv: A = chunks 2-5 (content head), B = chunks
        # 0,1,6,7 (pe + content tail); q: A = chunks 0-3, B = 4-7.
        KV_HALF_A = [2, 3, 4, 5]
        KV_HALF_B = [0, 1, 6, 7]
        Q_HALF_A = [0, 1, 2, 3]
        Q_HALF_B = [4, 5, 6, 7]
        cc_out = {}
        if use_ag:
            with tc.tile_pool(name="xpool", bufs=16) as xpool, \
                 tc.tile_pool(name="wstream", bufs=4) as wsp, \
                 tc.tile_pool(name="latstage", bufs=2) as lsp:
                xsb = []
                for cc in range(NCC):
                    t = xpool.tile([P, TQ], BF16, name=f"xsb{cc}", tag="xsb")
                    nc.sync.dma_start(out=t[:], in_=xT[cc * P:(cc + 1) * P, :])
                    xsb.append(t)
                for wname, wh, lbs in [
                    ("kva", wkv_a, KV_HALF_A), ("kvb", wkv_a, KV_HALF_B),
                    ("qa", wq_a, Q_HALF_A), ("qb", wq_a, Q_HALF_B),
                ]:
                    ccin = dram.tile([4 * P, TQ], BF16, name=f"cc_in_{wname}",
                                     tag=f"cc_in_{wname}")
                    ccout = dram.tile([16 * P, TQ], BF16, name=f"cc_out_{wname}",
                                      tag=f"cc_out_{wname}")
                    cc_out[wname] = ccout
                    lat = lsp.tile([P, 4, TQ], BF16, name=f"lat_{wname}", tag="lat")
                    pss = [ps_tile(f"lat_ps_{wname}{i}") for i in range(4)]
                    half = 0 if wname in ("kva", "qa") else 1
                    for cc in range(NCC):
                        # host permuted the weight columns into half order, so
                        # each half is one contiguous 512-column slab
                        wt = wsp.tile([P, 4 * P], BF16, name=f"wt_{wname}{cc}", tag="wt")
                        nc.sync.dma_start(
                            out=wt[:],
                            in_=wh[cc * P:(cc + 1) * P, half * 4 * P:(half + 1) * 4 * P])
                        for i in range(4):
                            nc.tensor.matmul(
                                pss[i][:], wt[:, i * P:(i + 1) * P], xsb[cc][:],
                                start=(cc == 0), stop=(cc == NCC - 1))
                    for i in range(4):
                        nc.scalar.copy(out=lat[:, i, :], in_=pss[i][:])
                    for i in range(4):
                        nc.sync.dma_start(out=ccin[i * P:(i + 1) * P, :], in_=lat[:, i, :])
                    nc.gpsimd.collective_compute(
                        "AllGather", mybir.AluOpType.bypass, replica_groups=RG,
                        ins=[ccin.opt()], outs=[ccout.opt()])

        def load_lat(pool, name, tt, half_a, half_b, names):
            # assemble the 8-chunk latent block for token-tile tt from the two
            # gathered halves (or compute locally when use_ag=False)
            t = pool.tile([P, NLB, TQ], BF16, name=name, tag=pool.name)
            for src_name, lbs in ((names[0], half_a), (names[1], half_b)):
                ccout = cc_out[src_name]
                blk = ccout[4 * P * tt:4 * P * (tt + 1), :].rearrange(
                    "(c p) t -> p c t", p=P)
                for i, lb in enumerate(lbs):
                    nc.sync.dma_start(out=t[:, lb, :], in_=blk[:, i, :])
            return t

        # ---- K/V sections ----
        def k_head(h, tt, kvsb_t, ss_k, kuns):
            kc_ps = ps_tile(f"kc_ps_{h}_{tt}")
            for j in range(CONTENT // P):
                nc.tensor.matmul(kc_ps[:], wkb_sb[:, j, h * D:(h + 1) * D],
                                 kvsb_t[:, 2 + j, :], start=(j == 0), stop=(j == 5))
            kpe_ps = ps_tile(f"kpe_ps_{h}_{tt}")
            for j in range(KV_PE // P):
                nc.tensor.matmul(kpe_ps[:], wkpe_sb[:, j, h * D:(h + 1) * D],
                                 kvsb_t[:, j, :], start=(j == 0), stop=(j == 1))
            hd = D // 2
            # kswap = halves of kpe swapped (PSUM reads may cross partitions)
            kswap = ropep.tile([P, TQ], BF16, name=f"kswap_{h}_{tt}", tag="rope")
            nc.scalar.copy(out=kswap[0:hd, :], in_=kpe_ps[hd:D, :])
            nc.scalar.copy(out=kswap[hd:D, :], in_=kpe_ps[0:hd, :])
            t1 = ropep.tile([P, TQ], BF16, name=f"t1_{h}_{tt}", tag="rope")
            nc.vector.tensor_mul(t1[:], kpe_ps[:], cos2_sb[:, tt * TQ:(tt + 1) * TQ])
            t2 = ropep.tile([P, TQ], BF16, name=f"t2_{h}_{tt}", tag="rope")
            nc.vector.tensor_mul(t2[:], kswap[:], sin2n_sb[:, tt * TQ:(tt + 1) * TQ])
            nc.vector.tensor_add(t1[:], t1[:], t2[:])
            k_un = tmpk.tile([P, TQ], BF16, name=f"k_un_{h}_{tt}", tag="k_un")
            nc.vector.tensor_add(k_un[:], t1[:], kc_ps[:])
            kuns.append(k_un)
            sq = tmpsq.tile([P, TQ], BF16, name=f"ksq_{h}_{tt}", tag="sq")
            nc.vector.tensor_mul(sq[:], k_un[:], k_un[:])
            row_mm(ss_k, h, ones_red[:], sq[:])

        def v_block(tt, t4, kvsb_t):
            v_ps = ps_tile(f"v_ps_{tt}_{t4}")
            for j in range(CONTENT // P):
                nc.tensor.matmul(v_ps[:], kvsb_t[:, 2 + j, t4 * P:(t4 + 1) * P],
                                 wv_sb[:, j, :], start=(j == 0), stop=(j == 5))
            nc.scalar.copy(out=v_sb[:, tt * 4 + t4, :], in_=v_ps[:])

        def norm_tail(tt, ss, scale, bias_t, srcs, dst, which):
            sroot = normf.tile([P, TQ], F32, name=f"sroot_{which}_{tt}", tag="nf")
            nc.scalar.activation(sroot[:], ss[:], AF.Sqrt, bias=bias_t[:], scale=scale)
            rinv = normf.tile([P, TQ], F32, name=f"rinv_{which}_{tt}", tag="nf")
            nc.vector.reciprocal(rinv[:], sroot[:])
            rbf = normb.tile([P, TQ], BF16, name=f"rbf_{which}_{tt}", tag="nb")
            nc.vector.tensor_copy(out=rbf[:], in_=rinv[:])
            for h in range(HG):
                bc = ps_tile(f"bc_{which}_{h}_{tt}")
                nc.tensor.matmul(bc[:], sels[h][:], rbf[:], start=True, stop=True)
                nc.vector.tensor_mul(dst[:, h, tt * TQ:(tt + 1) * TQ], srcs[h][:], bc[:])

        def q_sec(tt, qlsb_t, ss_q, qcs):
            qps = []
            for h in range(HG):
                q_ps = ps_tile(f"q_ps_{h}_{tt}")
                qps.append(q_ps)
                for j in range(NLB // 2):
                    nc.tensor.matmul(q_ps[:], wqb_sb[:, j, h * D:(h + 1) * D],
                                     qlsb_t[:, j, :], start=(j == 0), stop=False)
            for h in range(HG):
                q_ps = qps[h]
                for j in range(NLB // 2, NLB):
                    nc.tensor.matmul(q_ps[:], wqb_sb[:, j, h * D:(h + 1) * D],
                                     qlsb_t[:, j, :], start=False, stop=(j == NLB - 1))
                qc = castpool.tile([P, TQ], BF16, name=f"qc_{h}_{tt}", tag="cast")
                nc.scalar.copy(out=qc[:], in_=q_ps[:])
                qcs.append(qc)
                sq = tmpsq.tile([P, TQ], BF16, name=f"qsq_{h}_{tt}", tag="sq")
                nc.scalar.activation(sq[:], q_ps[:], AF.Square, bias=zeros128[:], scale=1.0)
                row_mm(ss_q, h, ones_red[:], sq[:])

        # ---- attention ----
        def a_tail(qb, den4, ycs):
            rinv = normf.tile([P, TQ], F32, name=f"rden_{qb}", tag="nf")
            nc.vector.reciprocal(rinv[:], den4[:])
            rbf = normb.tile([P, TQ], BF16, name=f"rdenb_{qb}", tag="nb")
            nc.vector.tensor_copy(out=rbf[:], in_=rinv[:])
            for h in range(HG):
                bc = ps_tile(f"abc_{h}_{qb}")
                nc.tensor.matmul(bc[:], sels[h][:], rbf[:], start=True, stop=True)
                nc.vector.tensor_mul(yTn_sb[:, h, qb * TQ:(qb + 1) * TQ],
                                     ycs[h][:], bc[:])

        pending_a = []

        def a_block(qb):
            # memset to 1.0 (not 0): unused rows go through reciprocal and
            # 1/0=inf would poison the selector matmul with 0*inf=NaN
            den4 = ps_tile(f"den4_{qb}")
            nc.vector.memset(den4[:], 1.0)
            ycs = []
            nkt = 4 * (qb + 1)
            for h in range(HG):
                yt_ps = ps_tile(f"yt_ps_{h}_{qb}")
                acc = accpool.tile([P, TQ], BF16, name=f"acc_{h}_{qb}", tag="acc")

                def emit_sc(kt):
                    sc_ps = ps_tile(f"sc_ps_{h}_{qb}_{kt}")
                    nc.tensor.matmul(sc_ps[:], kTn_sb[:, h, kt * P:(kt + 1) * P],
                                     qTn_sb[:, h, qb * TQ:(qb + 1) * TQ],
                                     start=True, stop=True)
                    ex = expool.tile([P, TQ], BF16, name=f"ex_{h}_{qb}_{kt}", tag="ex")
                    nc.scalar.activation(ex[:], sc_ps[:], AF.Exp,
                                         bias=zeros128[:], scale=1.0)
                    jrel = kt - 4 * qb
                    if jrel >= 0:
                        if jrel > 0:
                            nc.vector.memset(ex[:, 0:P * jrel], 0.0)
                        nc.vector.tensor_mul(ex[:, P * jrel:P * (jrel + 1)],
                                             ex[:, P * jrel:P * (jrel + 1)], tri_sb[:])
                    return ex

                def emit_pv(kt, ex):
                    if kt == 0:
                        nc.vector.tensor_copy(out=acc[:], in_=ex[:])
                    else:
                        nc.vector.tensor_add(acc[:], acc[:], ex[:])
                    nc.tensor.matmul(yt_ps[:], v_sb[:, kt, h * D:(h + 1) * D], ex[:],
                                     start=(kt == 0), stop=(kt == nkt - 1))

                # 2-deep lookahead: the score matmuls for kt+1/kt+2 are issued
                # before pv(kt), so the exp for each pv is ready when the PE
                # reaches it (PE is in-order)
                exs = {0: emit_sc(0)}
                if nkt > 1:
                    exs[1] = emit_sc(1)
                for kt in range(nkt):
                    if kt + 2 < nkt:
                        exs[kt + 2] = emit_sc(kt + 2)
                    emit_pv(kt, exs.pop(kt))
                row_mm(den4, h, ones_red[:], acc[:])
                yc = castpool.tile([P, TQ], BF16, name=f"yc_{h}_{qb}", tag="cast")
                nc.scalar.copy(out=yc[:], in_=yt_ps[:])
                ycs.append(yc)
                if pending_a and h == 1:
                    a_tail(*pending_a.pop(0))
            pending_a.append((qb, den4, ycs))

        # ---- emission: KV sections, then Q sections woven with A blocks ----
        with tc.tile_pool(name="kvpool", bufs=2) as kvpool, \
             tc.tile_pool(name="qlpool", bufs=2) as qlpool, \
             tc.tile_pool(name="xpool2", bufs=16) as xpool2, \
             tc.tile_pool(name="wstream2", bufs=3) as wsp2:

            def local_lat(pool, name, tt, wh, order):
                dst = pool.tile([P, NLB, TQ], BF16, name=name, tag=pool.name)
                xsb2 = []
                for cc in range(NCC):
                    t = xpool2.tile([P, TQ], BF16, name=f"x2_{name}_{cc}", tag="xsb2")
                    nc.sync.dma_start(out=t[:], in_=xT[cc * P:(cc + 1) * P,
                                                      tt * TQ:(tt + 1) * TQ])
                    xsb2.append(t)
                pss = [ps_tile(f"lat_ps_{name}_{lb}") for lb in range(NLB)]
                for cc in range(NCC):
                    wt = wsp2.tile([P, LORA], BF16, name=f"w2_{name}_{cc}", tag="wt2")
                    nc.sync.dma_start(out=wt[:], in_=wh[cc * P:(cc + 1) * P, :])
                    for lb in range(NLB):
                        nc.tensor.matmul(pss[lb][:], wt[:, lb * P:(lb + 1) * P],
                                         xsb2[cc][:], start=(cc == 0), stop=(cc == NCC - 1))
                for pos, lb in enumerate(order):
                    nc.scalar.copy(out=dst[:, lb, :], in_=pss[pos][:])
                return dst

            pending_k = []
            for tt in range(NTT):
                if use_ag:
                    kvsb_t = load_lat(kvpool, f"kvsb{tt}", tt, KV_HALF_A, KV_HALF_B,
                                      ("kva", "kvb"))
                else:
                    kvsb_t = local_lat(kvpool, f"kvsb{tt}", tt, wkv_a, KV_HALF_A + KV_HALF_B)
                ss_k = ps_tile(f"ss_k_{tt}")
                nc.vector.memset(ss_k[:], 1.0)
                kuns = []
                for h in range(HG):
                    k_head(h, tt, kvsb_t, ss_k, kuns)
                if pending_k:
                    p = pending_k.pop(0)
                    norm_tail(p[0], p[1], 1.0 / D, eps_k128, p[2], kTn_sb, "k")
                for t4 in range(4):
                    v_block(tt, t4, kvsb_t)
                pending_k.append((tt, ss_k, kuns))
            p = pending_k.pop(0)
            norm_tail(p[0], p[1], 1.0 / D, eps_k128, p[2], kTn_sb, "k")

            pending_q = []
            for tt in range(NTT):
                if use_ag:
                    qlsb_t = load_lat(qlpool, f"qlsb{tt}", tt, Q_HALF_A, Q_HALF_B,
                                      ("qa", "qb"))
                else:
                    qlsb_t = local_lat(qlpool, f"qlsb{tt}", tt, wq_a, Q_HALF_A + Q_HALF_B)
                ss_q = ps_tile(f"ss_q_{tt}")
                nc.vector.memset(ss_q[:], 1.0)
                qcs = []
                q_sec(tt, qlsb_t, ss_q, qcs)
                if pending_q:
                    p = pending_q.pop(0)
                    norm_tail(p[0], p[1], 1.0, eps_q128, p[2], qTn_sb, "q")
                    a_block(p[0])
                pending_q.append((tt, ss_q, qcs))
            p = pending_q.pop(0)
            norm_tail(p[0], p[1], 1.0, eps_q128, p[2], qTn_sb, "q")
            a_block(p[0])

        # ---- phase O: output projection; pass 1 (tt 0-2) can run while the
        # last attention block's ScalarE work drains, then tail + pass 2 ----
        attn_ctx.close()
        opool = ctx.enter_context(tc.tile_pool(name="opool", bufs=4))
        wop = ctx.enter_context(tc.tile_pool(name="wop", bufs=16))
        wo_ts = []
        for ct in range(C // P):
            wo_t = wop.tile([P, HG, P], BF16, name=f"wo_t{ct}", tag="wo_t")
            nc.scalar.dma_start(out=wo_t[:],
                              in_=wo[:, ct * P:(ct + 1) * P].rearrange("(h p) c -> p h c", p=P))
            wo_ts.append(None)
            ops = [ps_tile(f"o_ps_{ct}_{tt}") for tt in range(3)]
            for h in range(HG):
                for tt in range(3):
                    nc.tensor.matmul(ops[tt][:], wo_t[:, h, :],
                                     yTn_sb[:, h, tt * TQ:(tt + 1) * TQ],
                                     start=(h == 0), stop=(h == HG - 1))
            for tt in range(3):
                o_sb = opool.tile([P, TQ], BF16, name=f"o_sb_{ct}_{tt}", tag="o_sb")
                nc.vector.tensor_copy(out=o_sb[:], in_=ops[tt][:])
                nc.sync.dma_start(out=outT[ct * P:(ct + 1) * P, tt * TQ:(tt + 1) * TQ],
                                  in_=o_sb[:])
            if ct == 0:
                a_tail(*pending_a.pop(0))
            wo_ts[ct] = wo_t

        for ct in range(C // P):
            o_ps = ps_tile(f"o_ps3_{ct}")
            for h in range(HG):
                nc.tensor.matmul(o_ps[:], wo_ts[ct][:, h, :],
                                 yTn_sb[:, h, 3 * TQ:4 * TQ],
                                 start=(h == 0), stop=(h == HG - 1))
            o_sb = opool.tile([P, TQ], BF16, name=f"o_sb3_{ct}", tag="o_sb")
            nc.vector.tensor_copy(out=o_sb[:], in_=o_ps[:])
            nc.sync.dma_start(out=outT[ct * P:(ct + 1) * P, 3 * TQ:4 * TQ], in_=o_sb[:])

    nc.compile()
    return nc


def _get_nc(use_ag=USE_AG):
    if use_ag not in _NC_CACHE:
        _NC_CACHE[use_ag] = build_nc(use_ag)
    return _NC_CACHE[use_ag]


def _prepare_in_maps(x, cos, sin, wq_a, wq_b, wkv_a, wk_b, wkpe_b, wv_b, wo, use_ag=USE_AG):
    def bf(a):
        return np.ascontiguousarray(a).astype(NPBF16)

    cosT = np.asarray(cos, np.float32)[0, :, 0, :].T   # (64, T)
    sinT = np.asarray(sin, np.float32)[0, :, 0, :].T
    cos2 = bf(np.concatenate([cosT, cosT], axis=0))    # (128, T)
    sin2n = bf(np.concatenate([sinT, -sinT], axis=0))
    tri = (np.arange(P)[:, None] <= np.arange(P)[None, :]).astype(NPBF16)

    # permute latent-projection output columns into AllGather-half order so
    # the kernel streams contiguous 512-column slabs per half
    def perm_cols(w, halves):
        idx = np.concatenate([np.arange(c * P, (c + 1) * P) for half in halves for c in half])
        return np.ascontiguousarray(np.asarray(w, np.float32)[:, idx])
    KV_HALVES = ([2, 3, 4, 5], [0, 1, 6, 7])
    Q_HALVES = ([0, 1, 2, 3], [4, 5, 6, 7])
    wq_a_b = bf(perm_cols(wq_a, Q_HALVES))
    wkv_a_b = bf(perm_cols(wkv_a, KV_HALVES))
    wq_b_b, wk_b_b = bf(wq_b), bf(wk_b)
    wkpe_b_b, wv_b_b, wo_b = bf(wkpe_b), bf(wv_b), bf(wo)
    x = np.asarray(x, np.float32)

    in_maps = []
    for c in range(8):
        b, r = c // 4, c % 4
        if use_ag:
            xT_c = bf(x[b, r * TQ:(r + 1) * TQ, :].T)
        else:
            xT_c = bf(x[b].T)
        hgs = slice(r * HG * D, (r + 1) * HG * D)
        in_maps.append({
            "xT": xT_c,
            "wq_a": wq_a_b,
            "wkv_a": wkv_a_b,
            "wq_b": np.ascontiguousarray(wq_b_b[:, hgs]),
            "wk_b": np.ascontiguousarray(wk_b_b[:, hgs]),
            "wkpe_b": np.ascontiguousarray(wkpe_b_b[:, hgs]),
            "wv_b": np.ascontiguousarray(wv_b_b[:, hgs]),
            "wo": np.ascontiguousarray(wo_b[hgs, :]),
            "cos2": cos2,
            "sin2n": sin2n,
            "tri": tri,
        })
    return in_maps


def _assemble(results):
    out = np.empty((B, T, C), np.float32)
    for b in range(B):
        acc = results[4 * b]["outT"].astype(np.float32)
        for r in range(1, 4):
            acc = acc + results[4 * b + r]["outT"].astype(np.float32)
        out[b] = acc.T
    return out


def _run(inputs, use_ag=USE_AG, trace=False):
    nc = _get_nc(use_ag)
    in_maps = _prepare_in_maps(use_ag=use_ag, **inputs)
    res = run_bass_kernel_spmd(nc, in_maps, core_ids=list(range(8)), trace=trace)
    return _assemble(res.results), res


def kernel(**inputs):
    out, _ = _run(inputs)
    return out


# revision 13
# speedup vs baseline: 1.2638x; 1.0489x over previous
"""Trainium2 Bass kernel for MultiHeadLatentAttention (B=2, T=2048, C=2048, 16 heads).

Sharding over 8 NeuronCores: core c = (batch b = c//4, r = c%4).
 - Latent projections (x@wq_a, x@wkv_a) computed token-sharded (quarter r),
   in transposed layout (latent-dim on partitions), then AllGather-ed within
   each 4-core batch group (each gather split in two halves so consumers can
   start earlier).
 - Each core then handles head-group r (4 of 16 heads) for the full sequence:
   up-projections, RoPE+RMSNorm, block-causal attention, and a row-shard of
   the output projection.  Host sums the 4 partial outputs per batch.

All matmuls bf16 with fp32 PSUM accumulation.  RMS/softmax denominators are
ones-matmul partition reductions batched into 32-aligned PSUM rows (one DVE
reciprocal per four rows); per-row broadcasts are selector-matmuls.  Phases
are emitted interleaved (K/V sections, then Q sections with attention blocks
woven between them, then the output projection with the last attention block's
tail hidden under it) so the PE's in-order stream never waits on the
vector/scalar-engine chains and the ScalarE-bound softmax overlaps PE-dense
projections.
"""

from contextlib import ExitStack

import numpy as np
import ml_dtypes

import concourse.bass as bass
import concourse.tile as tile
import concourse.mybir as mybir
from concourse import bacc
from concourse.bass_utils import run_bass_kernel_spmd

BF16 = mybir.dt.bfloat16
F32 = mybir.dt.float32
NPBF16 = ml_dtypes.bfloat16
AF = mybir.ActivationFunctionType

P = 128
B, T, C = 2, 2048, 2048
H, D = 16, 128
LORA = 1024
KV_PE = 256           # latent rows 0-255 (chunks 0-1)
CONTENT = 768         # latent rows 256-1023 (chunks 2-7)
EPS = 1.1920929e-07
HG = 4                # heads per core
TQ = 512              # tokens per quarter / query block
NLB = LORA // P       # 8 latent row-blocks
NCC = C // P          # 16 contraction chunks of x
NTT = T // TQ         # 4 token 512-tiles
NKT = T // P          # 16 key tiles of 128
NQB = T // TQ         # 4 query blocks of 512
RG = [[0, 1, 2, 3], [4, 5, 6, 7]]

USE_AG = True

_NC_CACHE = {}


def build_nc(use_ag=USE_AG):
    nc = bacc.Bacc("TRN2", target_bir_lowering=False, debug=False, num_devices=8)

    xT = nc.dram_tensor("xT", [C, TQ if use_ag else T], BF16, kind="ExternalInput")
    wq_a = nc.dram_tensor("wq_a", [C, LORA], BF16, kind="ExternalInput")
    wkv_a = nc.dram_tensor("wkv_a", [C, LORA], BF16, kind="ExternalInput")
    wq_b = nc.dram_tensor("wq_b", [LORA, HG * D], BF16, kind="ExternalInput")
    wk_b = nc.dram_tensor("wk_b", [CONTENT, HG * D], BF16, kind="ExternalInput")
    wkpe_b = nc.dram_tensor("wkpe_b", [KV_PE, HG * D], BF16, kind="ExternalInput")
    wv_b = nc.dram_tensor("wv_b", [CONTENT, HG * D], BF16, kind="ExternalInput")
    wo = nc.dram_tensor("wo", [HG * D, C], BF16, kind="ExternalInput")
    # duplicated rope tables: cos2 = [cos; cos], sin2n = [sin; -sin]
    cos2 = nc.dram_tensor("cos2", [P, T], BF16, kind="ExternalInput")
    sin2n = nc.dram_tensor("sin2n", [P, T], BF16, kind="ExternalInput")
    tri = nc.dram_tensor("tri", [P, P], BF16, kind="ExternalInput")
    outT = nc.dram_tensor("outT", [C, T], BF16, kind="ExternalOutput")

    with tile.TileContext(nc) as tc, ExitStack() as ctx:
        dram = ctx.enter_context(tc.tile_pool(name="dram", bufs=1, space="DRAM"))
        psum = ctx.enter_context(tc.tile_pool(name="psum", bufs=8, space="PSUM"))
        consts = ctx.enter_context(tc.tile_pool(name="consts", bufs=1))
        persist = ctx.enter_context(tc.tile_pool(name="persist", bufs=1))
        tmpk = ctx.enter_context(tc.tile_pool(name="tmpk", bufs=8))
        ropep = ctx.enter_context(tc.tile_pool(name="ropep", bufs=6))
        tmpsq = ctx.enter_context(tc.tile_pool(name="tmpsq", bufs=4))
        normf = ctx.enter_context(tc.tile_pool(name="normf", bufs=3))
        normb = ctx.enter_context(tc.tile_pool(name="normb", bufs=4))
        expool = ctx.enter_context(tc.tile_pool(name="expool", bufs=6))
        accpool = ctx.enter_context(tc.tile_pool(name="accpool", bufs=4))
        castpool = ctx.enter_context(tc.tile_pool(name="castpool", bufs=18))

        def ps_tile(name):
            return psum.tile([P, 512], F32, name=name, tag="ps")

        def row_mm(out_tile, h, lhsT, rhs):
            # ones-matmul partition reduction into 32-aligned row 32*h.
            # Each row-MM is its own complete accumulation group: the rows are
            # disjoint so Tile sees no deps between them and may reorder; a
            # shared group would then accumulate onto stale bank bits.
            tp = (0, 32 * h) if h == 3 else None
            nc.tensor.matmul(out_tile[32 * h:32 * h + 1, :], lhsT, rhs,
                             start=True, stop=True, tile_position=tp)

        # ---- warm-up primer: ~4us of dense matmuls on locally-memset tiles
        # (no DMA deps) so the PE's HAM clock gate is released before the
        # first real matmuls arrive ----
        prime_sb = consts.tile([P, TQ], BF16, name="prime_sb")
        nc.vector.memset(prime_sb[:], 0.001)
        prime_w = consts.tile([P, P], BF16, name="prime_w")
        nc.vector.memset(prime_w[:], 0.001)
        _burst_n = [0]

        def warm_burst(n):
            # dep-free dense matmuls: re-promote the HAM clock gate while the
            # next section's AllGather dependency is still in flight
            _burst_n[0] += 1
            bp = ps_tile(f"warm_ps{_burst_n[0]}")
            for i in range(n):
                nc.tensor.matmul(bp[:], prime_w[:], prime_sb[:],
                                 start=(i == 0), stop=(i == n - 1))

        warm_burst(40)

        # ---- constants ----
        cos2_sb = consts.tile([P, T], BF16, name="cos2_sb")
        sin2n_sb = consts.tile([P, T], BF16, name="sin2n_sb")
        nc.scalar.dma_start(out=cos2_sb[:], in_=cos2[:])
        nc.scalar.dma_start(out=sin2n_sb[:], in_=sin2n[:])
        tri_sb = consts.tile([P, P], BF16, name="tri_sb")
        nc.scalar.dma_start(out=tri_sb[:], in_=tri[:])
        ones_red = consts.tile([P, 1], BF16, name="ones_red")
        nc.vector.memset(ones_red[:], 1.0)
        zeros128 = consts.tile([P, 1], F32, name="zeros128")
        nc.vector.memset(zeros128[:], 0.0)
        eps_k128 = consts.tile([P, 1], F32, name="eps_k128")
        nc.vector.memset(eps_k128[:], EPS)
        eps_q128 = consts.tile([P, 1], F32, name="eps_q128")
        nc.vector.memset(eps_q128[:], float(D) * EPS)
        sels = []
        for j in range(4):
            s = consts.tile([P, P], BF16, name=f"sel{j}")
            nc.vector.memset(s[:], 0.0)
            nc.vector.memset(s[32 * j:32 * j + 1, :], 1.0)
            sels.append(s)

        # ---- persistent phase products ----
        yTn_sb = persist.tile([P, HG, T], BF16, name="yTn_sb")
        # attention inputs live only until the last attention block; their
        # pool closes before phase O's pools open so the space is reused
        attn_ctx = ExitStack()
        attnp = attn_ctx.enter_context(tc.tile_pool(name="attnp", bufs=1))
        kTn_sb = attnp.tile([P, HG, T], BF16, name="kTn_sb")
        qTn_sb = attnp.tile([P, HG, T], BF16, name="qTn_sb")
        v_sb = attnp.tile([P, NKT, HG * D], BF16, name="v_sb")

        # up-projection weights (resident until end of Q sections)
        wu = attn_ctx.enter_context(tc.tile_pool(name="wu", bufs=1))
        wkb_sb = wu.tile([P, CONTENT // P, HG * D], BF16, name="wkb_sb")
        nc.scalar.dma_start(out=wkb_sb[:], in_=wk_b.rearrange("(j p) n -> p j n", p=P))
        wkpe_sb = wu.tile([P, KV_PE // P, HG * D], BF16, name="wkpe_sb")
        nc.scalar.dma_start(out=wkpe_sb[:], in_=wkpe_b.rearrange("(j p) n -> p j n", p=P))
        wv_sb = wu.tile([P, CONTENT // P, HG * D], BF16, name="wv_sb")
        nc.scalar.dma_start(out=wv_sb[:], in_=wv_b.rearrange("(j p) n -> p j n", p=P))
        wqb_sb = wu.tile([P, NLB, HG * D], BF16, name="wqb_sb")
        nc.scalar.dma_start(out=wqb_sb[:], in_=wq_b.rearrange("(j p) n -> p j n", p=P))

        # ---- phase L: latent projections + (halved) AllGathers ----
        # halves by latent rows: kv: A = chunks 2-5 (content head), B = chunks
        # 0,1,6,7 (pe + content tail); q: A = chunks 0-3, B = 4-7.
        KV_HALF_A = [2, 3, 4, 5]
        KV_HALF_B = [0, 1, 6, 7]
        Q_HALF_A = [0, 1, 2, 3]
        Q_HALF_B = [4, 5, 6, 7]
        cc_out = {}
        if use_ag:
            with tc.tile_pool(name="xpool", bufs=16) as xpool, \
                 tc.tile_pool(name="wstream", bufs=6) as wsp, \
                 tc.tile_pool(name="latstage", bufs=2) as lsp:
                xsb = []
                for cc in range(NCC):
                    t = xpool.tile([P, TQ], BF16, name=f"xsb{cc}", tag="xsb")
                    nc.sync.dma_start(out=t[:], in_=xT[cc * P:(cc + 1) * P, :])
                    xsb.append(t)
                for wname, wh, lbs in [
                    ("kva", wkv_a, KV_HALF_A), ("kvb", wkv_a, KV_HALF_B),
                    ("qa", wq_a, Q_HALF_A), ("qb", wq_a, Q_HALF_B),
                ]:
                    ccin = dram.tile([4 * P, TQ], BF16, name=f"cc_in_{wname}",
                                     tag=f"cc_in_{wname}")
                    ccout = dram.tile([16 * P, TQ], BF16, name=f"cc_out_{wname}",
                                      tag=f"cc_out_{wname}")
                    cc_out[wname] = ccout
                    lat = lsp.tile([P, 4, TQ], BF16, name=f"lat_{wname}", tag="lat")
                    pss = [ps_tile(f"lat_ps_{wname}{i}") for i in range(4)]
                    half = 0 if wname in ("kva", "qa") else 1
                    for cc in range(NCC):
                        # host permuted the weight columns into half order, so
                        # each half is one contiguous 512-column slab
                        wt = wsp.tile([P, 4 * P], BF16, name=f"wt_{wname}{cc}", tag="wt")
                        nc.sync.dma_start(
                            out=wt[:],
                            in_=wh[cc * P:(cc + 1) * P, half * 4 * P:(half + 1) * 4 * P])
                        for i in range(4):
                            nc.tensor.matmul(
                                pss[i][:], wt[:, i * P:(i + 1) * P], xsb[cc][:],
                                start=(cc == 0), stop=(cc == NCC - 1))
                    for i in range(4):
                        nc.scalar.copy(out=lat[:, i, :], in_=pss[i][:])
                    for i in range(4):
                        nc.sync.dma_start(out=ccin[i * P:(i + 1) * P, :], in_=lat[:, i, :])
                    nc.gpsimd.collective_compute(
                        "AllGather", mybir.AluOpType.bypass, replica_groups=RG,
                        ins=[ccin.opt()], outs=[ccout.opt()])

        def load_lat(pool, name, tt, half_a, half_b, names):
            # assemble the 8-chunk latent block for token-tile tt from the two
            # gathered halves (or compute locally when use_ag=False)
            t = pool.tile([P, NLB, TQ], BF16, name=name, tag=pool.name)
            for src_name, lbs in ((names[0], half_a), (names[1], half_b)):
                ccout = cc_out[src_name]
                blk = ccout[4 * P * tt:4 * P * (tt + 1), :].rearrange(
                    "(c p) t -> p c t", p=P)
                for i, lb in enumerate(lbs):
                    nc.sync.dma_start(out=t[:, lb, :], in_=blk[:, i, :])
            return t

        # ---- K/V sections ----
        def k_head(h, tt, kvsb_t, ss_k, kuns):
            kc_ps = ps_tile(f"kc_ps_{h}_{tt}")
            for j in range(CONTENT // P):
                nc.tensor.matmul(kc_ps[:], wkb_sb[:, j, h * D:(h + 1) * D],
                                 kvsb_t[:, 2 + j, :], start=(j == 0), stop=(j == 5))
            kpe_ps = ps_tile(f"kpe_ps_{h}_{tt}")
            for j in range(KV_PE // P):
                nc.tensor.matmul(kpe_ps[:], wkpe_sb[:, j, h * D:(h + 1) * D],
                                 kvsb_t[:, j, :], start=(j == 0), stop=(j == 1))
            hd = D // 2
            # kswap = halves of kpe swapped (PSUM reads may cross partitions)
            kswap = ropep.tile([P, TQ], BF16, name=f"kswap_{h}_{tt}", tag="rope")
            nc.scalar.copy(out=kswap[0:hd, :], in_=kpe_ps[hd:D, :])
            nc.scalar.copy(out=kswap[hd:D, :], in_=kpe_ps[0:hd, :])
            t1 = ropep.tile([P, TQ], BF16, name=f"t1_{h}_{tt}", tag="rope")
            nc.vector.tensor_mul(t1[:], kpe_ps[:], cos2_sb[:, tt * TQ:(tt + 1) * TQ])
            t2 = ropep.tile([P, TQ], BF16, name=f"t2_{h}_{tt}", tag="rope")
            nc.vector.tensor_mul(t2[:], kswap[:], sin2n_sb[:, tt * TQ:(tt + 1) * TQ])
            nc.vector.tensor_add(t1[:], t1[:], t2[:])
            k_un = tmpk.tile([P, TQ], BF16, name=f"k_un_{h}_{tt}", tag="k_un")
            nc.vector.tensor_add(k_un[:], t1[:], kc_ps[:])
            kuns.append(k_un)
            sq = tmpsq.tile([P, TQ], BF16, name=f"ksq_{h}_{tt}", tag="sq")
            nc.vector.tensor_mul(sq[:], k_un[:], k_un[:])
            row_mm(ss_k, h, ones_red[:], sq[:])

        def v_block(tt, t4, kvsb_t):
            v_ps = ps_tile(f"v_ps_{tt}_{t4}")
            for j in range(CONTENT // P):
                nc.tensor.matmul(v_ps[:], kvsb_t[:, 2 + j, t4 * P:(t4 + 1) * P],
                                 wv_sb[:, j, :], start=(j == 0), stop=(j == 5))
            nc.scalar.copy(out=v_sb[:, tt * 4 + t4, :], in_=v_ps[:])

        def norm_tail(tt, ss, scale, bias_t, srcs, dst, which):
            sroot = normf.tile([P, TQ], F32, name=f"sroot_{which}_{tt}", tag="nf")
            nc.scalar.activation(sroot[:], ss[:], AF.Sqrt, bias=bias_t[:], scale=scale)
            rinv = normf.tile([P, TQ], F32, name=f"rinv_{which}_{tt}", tag="nf")
            nc.vector.reciprocal(rinv[:], sroot[:])
            rbf = normb.tile([P, TQ], BF16, name=f"rbf_{which}_{tt}", tag="nb")
            nc.vector.tensor_copy(out=rbf[:], in_=rinv[:])
            for h in range(HG):
                bc = ps_tile(f"bc_{which}_{h}_{tt}")
                nc.tensor.matmul(bc[:], sels[h][:], rbf[:], start=True, stop=True)
                nc.vector.tensor_mul(dst[:, h, tt * TQ:(tt + 1) * TQ], srcs[h][:], bc[:])

        def q_sec(tt, qlsb_t, ss_q, qcs):
            qps = []
            for h in range(HG):
                q_ps = ps_tile(f"q_ps_{h}_{tt}")
                qps.append(q_ps)
                for j in range(NLB // 2):
                    nc.tensor.matmul(q_ps[:], wqb_sb[:, j, h * D:(h + 1) * D],
                                     qlsb_t[:, j, :], start=(j == 0), stop=False)
            for h in range(HG):
                q_ps = qps[h]
                for j in range(NLB // 2, NLB):
                    nc.tensor.matmul(q_ps[:], wqb_sb[:, j, h * D:(h + 1) * D],
                                     qlsb_t[:, j, :], start=False, stop=(j == NLB - 1))
                qc = castpool.tile([P, TQ], BF16, name=f"qc_{h}_{tt}", tag="cast")
                nc.scalar.copy(out=qc[:], in_=q_ps[:])
                qcs.append(qc)
                sq = tmpsq.tile([P, TQ], BF16, name=f"qsq_{h}_{tt}", tag="sq")
                nc.scalar.activation(sq[:], q_ps[:], AF.Square, bias=zeros128[:], scale=1.0)
                row_mm(ss_q, h, ones_red[:], sq[:])

        # ---- attention ----
        def a_tail(qb, den4, ycs):
            rinv = normf.tile([P, TQ], F32, name=f"rden_{qb}", tag="nf")
            nc.vector.reciprocal(rinv[:], den4[:])
            rbf = normb.tile([P, TQ], BF16, name=f"rdenb_{qb}", tag="nb")
            nc.vector.tensor_copy(out=rbf[:], in_=rinv[:])
            for h in range(HG):
                bc = ps_tile(f"abc_{h}_{qb}")
                nc.tensor.matmul(bc[:], sels[h][:], rbf[:], start=True, stop=True)
                nc.vector.tensor_mul(yTn_sb[:, h, qb * TQ:(qb + 1) * TQ],
                                     ycs[h][:], bc[:])

        pending_a = []

        def a_block(qb):
            # memset to 1.0 (not 0): unused rows go through reciprocal and
            # 1/0=inf would poison the selector matmul with 0*inf=NaN
            den4 = ps_tile(f"den4_{qb}")
            nc.vector.memset(den4[:], 1.0)
            ycs = []
            nkt = 4 * (qb + 1)
            for h in range(HG):
                yt_ps = ps_tile(f"yt_ps_{h}_{qb}")
                acc = accpool.tile([P, TQ], BF16, name=f"acc_{h}_{qb}", tag="acc")

                def emit_sc(kt):
                    sc_ps = ps_tile(f"sc_ps_{h}_{qb}_{kt}")
                    nc.tensor.matmul(sc_ps[:], kTn_sb[:, h, kt * P:(kt + 1) * P],
                                     qTn_sb[:, h, qb * TQ:(qb + 1) * TQ],
                                     start=True, stop=True)
                    ex = expool.tile([P, TQ], BF16, name=f"ex_{h}_{qb}_{kt}", tag="ex")
                    nc.scalar.activation(ex[:], sc_ps[:], AF.Exp,
                                         bias=zeros128[:], scale=1.0)
                    jrel = kt - 4 * qb
                    if jrel >= 0:
                        if jrel > 0:
                            nc.vector.memset(ex[:, 0:P * jrel], 0.0)
                        nc.vector.tensor_mul(ex[:, P * jrel:P * (jrel + 1)],
                                             ex[:, P * jrel:P * (jrel + 1)], tri_sb[:])
                    return ex

                def emit_pv(kt, ex):
                    if kt == 0:
                        nc.vector.tensor_copy(out=acc[:], in_=ex[:])
                    else:
                        nc.vector.tensor_add(acc[:], acc[:], ex[:])
                    nc.tensor.matmul(yt_ps[:], v_sb[:, kt, h * D:(h + 1) * D], ex[:],
                                     start=(kt == 0), stop=(kt == nkt - 1))

                # 2-deep lookahead: the score matmuls for kt+1/kt+2 are issued
                # before pv(kt), so the exp for each pv is ready when the PE
                # reaches it (PE is in-order)
                exs = {0: emit_sc(0)}
                if nkt > 1:
                    exs[1] = emit_sc(1)
                for kt in range(nkt):
                    if kt + 2 < nkt:
                        exs[kt + 2] = emit_sc(kt + 2)
                    emit_pv(kt, exs.pop(kt))
                row_mm(den4, h, ones_red[:], acc[:])
                yc = castpool.tile([P, TQ], BF16, name=f"yc_{h}_{qb}", tag="cast")
                nc.scalar.copy(out=yc[:], in_=yt_ps[:])
                ycs.append(yc)
                if pending_a and h == 1:
                    a_tail(*pending_a.pop(0))
            pending_a.append((qb, den4, ycs))

        # ---- emission: KV sections, then Q sections woven with A blocks ----
        with tc.tile_pool(name="kvpool", bufs=2) as kvpool, \
             tc.tile_pool(name="qlpool", bufs=2) as qlpool, \
             tc.tile_pool(name="xpool2", bufs=16) as xpool2, \
             tc.tile_pool(name="wstream2", bufs=3) as wsp2:

            def local_lat(pool, name, tt, wh, order):
                dst = pool.tile([P, NLB, TQ], BF16, name=name, tag=pool.name)
                xsb2 = []
                for cc in range(NCC):
                    t = xpool2.tile([P, TQ], BF16, name=f"x2_{name}_{cc}", tag="xsb2")
                    nc.sync.dma_start(out=t[:], in_=xT[cc * P:(cc + 1) * P,
                                                      tt * TQ:(tt + 1) * TQ])
                    xsb2.append(t)
                pss = [ps_tile(f"lat_ps_{name}_{lb}") for lb in range(NLB)]
                for cc in range(NCC):
                    wt = wsp2.tile([P, LORA], BF16, name=f"w2_{name}_{cc}", tag="wt2")
                    nc.sync.dma_start(out=wt[:], in_=wh[cc * P:(cc + 1) * P, :])
                    for lb in range(NLB):
                        nc.tensor.matmul(pss[lb][:], wt[:, lb * P:(lb + 1) * P],
                                         xsb2[cc][:], start=(cc == 0), stop=(cc == NCC - 1))
                for pos, lb in enumerate(order):
                    nc.scalar.copy(out=dst[:, lb, :], in_=pss[pos][:])
                return dst

            warm_burst(20)
            pending_k = []
            for tt in range(NTT):
                if use_ag:
                    kvsb_t = load_lat(kvpool, f"kvsb{tt}", tt, KV_HALF_A, KV_HALF_B,
                                      ("kva", "kvb"))
                else:
                    kvsb_t = local_lat(kvpool, f"kvsb{tt}", tt, wkv_a, KV_HALF_A + KV_HALF_B)
                ss_k = ps_tile(f"ss_k_{tt}")
                nc.vector.memset(ss_k[:], 1.0)
                kuns = []
                for h in range(HG):
                    k_head(h, tt, kvsb_t, ss_k, kuns)
                if pending_k:
                    p = pending_k.pop(0)
                    norm_tail(p[0], p[1], 1.0 / D, eps_k128, p[2], kTn_sb, "k")
                for t4 in range(4):
                    v_block(tt, t4, kvsb_t)
                pending_k.append((tt, ss_k, kuns))
            p = pending_k.pop(0)
            norm_tail(p[0], p[1], 1.0 / D, eps_k128, p[2], kTn_sb, "k")

            warm_burst(20)
            pending_q = []
            for tt in range(NTT):
                if use_ag:
                    qlsb_t = load_lat(qlpool, f"qlsb{tt}", tt, Q_HALF_A, Q_HALF_B,
                                      ("qa", "qb"))
                else:
                    qlsb_t = local_lat(qlpool, f"qlsb{tt}", tt, wq_a, Q_HALF_A + Q_HALF_B)
                ss_q = ps_tile(f"ss_q_{tt}")
                nc.vector.memset(ss_q[:], 1.0)
                qcs = []
                q_sec(tt, qlsb_t, ss_q, qcs)
                if pending_q:
                    p = pending_q.pop(0)
                    norm_tail(p[0], p[1], 1.0, eps_q128, p[2], qTn_sb, "q")
                    a_block(p[0])
                pending_q.append((tt, ss_q, qcs))
            p = pending_q.pop(0)
            norm_tail(p[0], p[1], 1.0, eps_q128, p[2], qTn_sb, "q")
            a_block(p[0])

        # ---- phase O: output projection; pass 1 (tt 0-2) can run while the
        # last attention block's ScalarE work drains, then tail + pass 2 ----
        attn_ctx.close()
        opool = ctx.enter_context(tc.tile_pool(name="opool", bufs=4))
        wop = ctx.enter_context(tc.tile_pool(name="wop", bufs=16))
        wo_ts = []
        for ct in range(C // P):
            wo_t = wop.tile([P, HG, P], BF16, name=f"wo_t{ct}", tag="wo_t")
            nc.scalar.dma_start(out=wo_t[:],
                              in_=wo[:, ct * P:(ct + 1) * P].rearrange("(h p) c -> p h c", p=P))
            wo_ts.append(None)
            ops = [ps_tile(f"o_ps_{ct}_{tt}") for tt in range(3)]
            for h in range(HG):
                for tt in range(3):
                    nc.tensor.matmul(ops[tt][:], wo_t[:, h, :],
                                     yTn_sb[:, h, tt * TQ:(tt + 1) * TQ],
                                     start=(h == 0), stop=(h == HG - 1))
            for tt in range(3):
                o_sb = opool.tile([P, TQ], BF16, name=f"o_sb_{ct}_{tt}", tag="o_sb")
                nc.vector.tensor_copy(out=o_sb[:], in_=ops[tt][:])
                nc.sync.dma_start(out=outT[ct * P:(ct + 1) * P, tt * TQ:(tt + 1) * TQ],
                                  in_=o_sb[:])
            if ct == 0:
                a_tail(*pending_a.pop(0))
            wo_ts[ct] = wo_t

        for ct in range(C // P):
            o_ps = ps_tile(f"o_ps3_{ct}")
            for h in range(HG):
                nc.tensor.matmul(o_ps[:], wo_ts[ct][:, h, :],
                                 yTn_sb[:, h, 3 * TQ:4 * TQ],
                                 start=(h == 0), stop=(h == HG - 1))
            o_sb = opool.tile([P, TQ], BF16, name=f"o_sb3_{ct}", tag="o_sb")
            nc.vector.tensor_copy(out=o_sb[:], in_=o_ps[:])
            nc.sync.dma_start(out=outT[ct * P:(ct + 1) * P, 3 * TQ:4 * TQ], in_=o_sb[:])

    nc.compile()
    return nc


def _get_nc(use_ag=USE_AG):
    if use_ag not in _NC_CACHE:
        _NC_CACHE[use_ag] = build_nc(use_ag)
    return _NC_CACHE[use_ag]


def _prepare_in_maps(x, cos, sin, wq_a, wq_b, wkv_a, wk_b, wkpe_b, wv_b, wo, use_ag=USE_AG):
    def bf(a):
        return np.ascontiguousarray(a).astype(NPBF16)

    cosT = np.asarray(cos, np.float32)[0, :, 0, :].T   # (64, T)
    sinT = np.asarray(sin, np.float32)[0, :, 0, :].T
    cos2 = bf(np.concatenate([cosT, cosT], axis=0))    # (128, T)
    sin2n = bf(np.concatenate([sinT, -sinT], axis=0))
    tri = (np.arange(P)[:, None] <= np.arange(P)[None, :]).astype(NPBF16)

    # permute latent-projection output columns into AllGather-half order so
    # the kernel streams contiguous 512-column slabs per half
    def perm_cols(w, halves):
        idx = np.concatenate([np.arange(c * P, (c + 1) * P) for half in halves for c in half])
        return np.ascontiguousarray(np.asarray(w, np.float32)[:, idx])
    KV_HALVES = ([2, 3, 4, 5], [0, 1, 6, 7])
    Q_HALVES = ([0, 1, 2, 3], [4, 5, 6, 7])
    wq_a_b = bf(perm_cols(wq_a, Q_HALVES))
    wkv_a_b = bf(perm_cols(wkv_a, KV_HALVES))
    wq_b_b, wk_b_b = bf(wq_b), bf(wk_b)
    wkpe_b_b, wv_b_b, wo_b = bf(wkpe_b), bf(wv_b), bf(wo)
    x = np.asarray(x, np.float32)

    in_maps = []
    for c in range(8):
        b, r = c // 4, c % 4
        if use_ag:
            xT_c = bf(x[b, r * TQ:(r + 1) * TQ, :].T)
        else:
            xT_c = bf(x[b].T)
        hgs = slice(r * HG * D, (r + 1) * HG * D)
        in_maps.append({
            "xT": xT_c,
            "wq_a": wq_a_b,
            "wkv_a": wkv_a_b,
            "wq_b": np.ascontiguousarray(wq_b_b[:, hgs]),
            "wk_b": np.ascontiguousarray(wk_b_b[:, hgs]),
            "wkpe_b": np.ascontiguousarray(wkpe_b_b[:, hgs]),
            "wv_b": np.ascontiguousarray(wv_b_b[:, hgs]),
            "wo": np.ascontiguousarray(wo_b[hgs, :]),
            "cos2": cos2,
            "sin2n": sin2n,
            "tri": tri,
        })
    return in_maps


def _assemble(results):
    out = np.empty((B, T, C), np.float32)
    for b in range(B):
        acc = results[4 * b]["outT"].astype(np.float32)
        for r in range(1, 4):
            acc = acc + results[4 * b + r]["outT"].astype(np.float32)
        out[b] = acc.T
    return out


def _run(inputs, use_ag=USE_AG, trace=False):
    nc = _get_nc(use_ag)
    in_maps = _prepare_in_maps(use_ag=use_ag, **inputs)
    res = run_bass_kernel_spmd(nc, in_maps, core_ids=list(range(8)), trace=trace)
    return _assemble(res.results), res


def kernel(**inputs):
    out, _ = _run(inputs)
    return out
